# revision 40
# baseline (speedup 1.0000x reference)
"""Multi-head attention (ViT-style, N=1025 tokens incl. cls) on 8 TRN2 NeuronCores.

Reference semantics: the "separate cls-token attention" branch of the reference
is mathematically identical to row 0 of standard attention (same logits, same
softmax, same values), so the output is exactly
    out = softmax(Q K^T * hd^-0.5) V -> proj -> + bias.

Sharding: data-parallel over batch: B=16 -> 2 batches per core, weights
replicated, no collectives. ~422us HW exec on silicon (run variance
~+/-5us), rel err ~3.3e-3.

Changes vs the ~470us predecessor:
  - normalization multiplies ride the (otherwise idle) GpSimd engine
    mid-kernel, so the in-order DVE stream never blocks lin-filler psum
    drains (those stalls reset the PE DVFS p-state: a blocked PE restarts
    at 1.2GHz and needs ~3us of continuous work to regain 2.4GHz)
  - ALL normalizations compute 1/s as exp(-ln s) on ACT (2 x ~1.2us; the
    one loaded table set holds both Exp and Ln) -- the 6.7us DVE
    reciprocal sat in the in-order DVE stream exactly where psO/filler
    psum drains queue behind it, starving the PE into HAM clock drops
  - split-K proj for batch 1: the k=0..3 partial (independent of the
    final norm round) runs as round-4/5 filler, partials parked bf16 in
    dead qk slots; the serial tail is only k=4..5 + one combine-add per
    piece (phase-B psums ride the by-then-free ps_s pool: both e-chunks
    in one [128,1024] tile, one DVE add per piece)
  - input DMAs ordered by first use (the small m=0/m=6 wqk slices FIRST,
    then xT split at the batch boundary so linqk's first windows start at
    ~14us instead of ~19us; wv next, rest deferred), with dispatches
    alternating sync/scalar HW queues (each dispatch is ~0.6us of serial
    engine-queue time) and the two wqk m-slice blocks merged into one
    3D-AP DMA; lin_head emission follows DMA landing order
  - a dummy exp at t~0 pulls the ~2.7us ACT_TABLE_LOAD into the
    input-DMA window (the early-started PE otherwise waits for it at
    round 0's first exp)
  - tail R-broadcast + output-DMA dispatches alternate sync/scalar HW
    queues; the final norm's 8 big muls are load-balanced 3:5 over
    GpSimd (~0.83us/op) vs DVE (~0.4us/op)
  - a ~5us screen of dependency-free matmuls covers the tail norm chain
    so the HAM doesn't halve the PE clock right before the k=4..5 finals
  - proj(0) fillers drain 4-per-window during the last head pair instead of
    piling into the serial tail
  - per-(b,j) proj outputs accumulate into one [128,768] tile and ship in a
    single DMA (half the output-DMA dispatch cost on the sync engine)
Failed experiments (measured SLOWER, do not retry):
  - cls-pass S matmuls kc-major (pairing the two row-tile bases into the
    same single-bank psum): DEVICE HANG -- concurrent row-tile halves
    must target different PSUM banks (main S pairs do)
  - xT DMAs split by 512-col windows: completion-sem waits coarsen to
    "#64 on this ring", first linqk waited ~25us instead of ~19
  - round-boundary junk screens (KJUNKR=3) + longer tail screen
    (KJUNK=44): +1.4us
  - fp8 DoubleRow S-matmuls: on silicon a DR matmul streams 2F moving
    elements (fold dim rides the moving stream), costing the same as the two
    bf16 row-tiled matmuls it replaces -- plus cast/fold overhead
  - balanced 9x~114 token chunks: stationary operands lose 256B alignment
    -> LDWEIGHTS penalty, ~+90us
  - software-pipelining O one chunk behind S, and JIT V-piece emission:
    both shuffle work into the DVE stream at points that block psum drains
  - reciprocal_approx_fast / gpsimd-issued DMAs: unsupported by this
    walrus/runtime (ISA-wrong-length / device hang)
  - reserving proj(0) fillers for the tail norm-chain window: +14us (x2)
  - splitting the mid-kernel DVE reciprocal into halves: +13us solo (the
    smdd DMA needs both halves, so the chain got longer, not shorter)
  - rp pool 5-6 bufs: SBUF overflow (no per-partition headroom left)
  - PE warmup junk matmuls during the input-DMA wait: neutral within noise
  - xT input DMAs on the scalar-engine queue: slightly slower

Per-core layout strategy (matmul operands bf16, f32 PSUM accumulation):
  - Host pre-transposes x / weights so contraction dims land on partitions.
  - qkT = wqkT.T @ xT      -> [1536, tok]  (Q^T,K^T: head dim on partitions)
  - V   = xT.T @ wvT       -> [tok, 768] in 65-stride head layout with a
    ones column per head (softmax sums ride the O matmul for free)
  - S^T = K_h^T.T @ Q_h^T  -> [ktok, qtok], two heads row-tiled concurrently
    (tile_position from base partitions 0/64); query windows 2x512, the last
    query column batched per head pair into a [128, 18] collector
  - P^T = exp(S^T * scale) on ScalarE, one [128, 1024] instr per k-chunk
    (ACT costs (N+352) cycles -> wide instrs; no max-subtraction needed
    since |logits| < ~4 for this distribution)
  - O^T = Vaug_h.T @ P^T   -> [65, qtok] PSUM; row 64 = softmax sums
  - unnormalized O^T is cast straight into xstdT (bf16); sums are collected
    into partition-aligned batch tiles (rows 0/32/64/96), one wide
    reciprocal per 4 sites (DVE mid-kernel, ACT exp(-ln) at the tail),
    DRAM-roundtrip partition-broadcast, then in-place multiply on GpSimd
    (no engine can broadcast across partitions; DMA can, from DRAM)
  - y = xstdT.T @ pwT + bias -> [tok, 768] -> bf16 out DMA (host casts f32)

Emission order doubles as the static-schedule priority (Tile list-scheduler):
attention leads, LIN-QK/LIN-V/proj pieces are queued as fillers drained
between attention stages so they soak up PE idle during the ACT-paced
attention pipeline.

Post-scheduling passes (this walrus allows ONE sync wait per engine
instruction): standalone LDWEIGHTS are re-fused into matmuls, then excess
waits are hoisted onto single-wait PE NoOps (semaphores are monotonic and
each sequencer executes in order, so earlier-covered waits are dropped).
"""

import os

import numpy as np
import ml_dtypes

import concourse.bass as bass
import concourse.mybir as mybir
import concourse.tile as tile

# optimization gates (bisect switches)
OPT_DMA = os.environ.get("KOPT_DMA", "1") == "1"
OPT_LNEXP = os.environ.get("KOPT_LNEXP", "1") == "1"

NCORES = 8
B, N, C = 16, 1025, 768
NB = B // NCORES          # batches per core
H = 12                    # heads
HD = C // H               # 64
HP = H // 2               # head pairs
TOK = NB * N              # tokens per core (2050)
SCALE = float(HD) ** -0.5
DQK = 2 * C               # 1536
F32 = mybir.dt.float32
BF16 = mybir.dt.bfloat16
Exp = mybir.ActivationFunctionType.Exp

# per-batch token chunks (attention / V / proj tiling): 8 x 128 + 1.
# NOTE: keep 128-chunks — a balanced 9 x ~114 split measured ~90us SLOWER
# (stationary operand offsets lose 256B alignment -> LDWEIGHTS penalty)
TCH = [(j * 128, 128) for j in range(8)] + [(1024, 1)]
# query-token windows (PSUM bank = 512 f32); last column handled in batched pass
QW = [(0, 512), (512, 512)]


def bcast_rows(ap_row, nrows):
    """AP reading one [1, n] row replicated across nrows partitions."""
    return bass.AP(
        tensor=ap_row.tensor,
        offset=ap_row.offset,
        ap=[[0, nrows]] + list(ap_row.ap[1:]),
    )


def two_blocks(ap2d, c0, w, c1):
    """3D AP over two equal-width column blocks [c0:c0+w] and [c1:c1+w] of a
    [P, ...] 2D AP -- lets one DMA cover both wqk m-slices."""
    pdim, (cstride, _) = ap2d.ap[0], ap2d.ap[1]
    return bass.AP(
        tensor=ap2d.tensor,
        offset=ap2d.offset + c0 * cstride,
        ap=[list(pdim), [(c1 - c0) * cstride, 2], [cstride, w]],
    )


def build_nc():
    nc = bass.Bass()
    xT_e = nc.declare_dram_parameter("xT", [C, TOK], BF16, isOutput=False)
    wqk_e = nc.declare_dram_parameter("wqkT", [C, DQK], BF16, isOutput=False)
    wv_e = nc.declare_dram_parameter("wvT", [C, C], BF16, isOutput=False)
    pw_e = nc.declare_dram_parameter("pwT", [C, C], BF16, isOutput=False)
    pb_e = nc.declare_dram_parameter("pb", [C], F32, isOutput=False)
    out_e = nc.declare_dram_parameter("out", [TOK, C], BF16, isOutput=True)

    with tile.TileContext(nc) as tc:
        with (
            tc.tile_pool(name="big", bufs=1) as big,
            tc.tile_pool(name="ps_lin", bufs=2, space="PSUM") as ps_lin,
            tc.tile_pool(name="ps_s", bufs=2, space="PSUM") as ps_s,
            tc.tile_pool(name="ps_o", bufs=2, space="PSUM") as ps_o,
            tc.tile_pool(name="ptp", bufs=4) as ptp,
            tc.tile_pool(name="rp", bufs=3) as rp,
            tc.tile_pool(name="smtp", bufs=6) as smtp,
            tc.tile_pool(name="dr", bufs=6, space="DRAM") as dr,
            tc.tile_pool(name="outp", bufs=3) as outp,
        ):
            # ---- persistent SBUF tensors (static: one slot per tag) ----
            def big_tile(shape, dtype, nm):
                return big.tile(shape, dtype, tag=nm, name=nm)

            xT = [big_tile([128, TOK], BF16, f"xT{k}") for k in range(6)]


            wqk = [big_tile([128, DQK], BF16, f"wqk{k}") for k in range(6)]
            wv = [big_tile([128, C], BF16, f"wv{k}") for k in range(6)]
            pw = [big_tile([128, C], BF16, f"pw{k}") for k in range(6)]
            pb = big_tile([128, C], F32, "pb")
            # Q^T|K^T chunks: m 0..5 = Q (heads 2m,2m+1), 6..11 = K
            qk = [big_tile([128, TOK], BF16, f"qk{m}") for m in range(12)]
            # V with 65-stride head layout (col 64 of each head block = ones)
            vaug = [[big_tile([128, 65 * H], BF16, f"vaug{b}_{j}")
                     for j in range(9)] for b in range(NB)]
            # attention output transposed, per c-chunk (= head pair)
            xstdT = [[big_tile([128, N], BF16, f"xstdT{b}_{k}")
                      for k in range(6)] for b in range(NB)]
            # ---- ACT warmup ----
            # walrus inserts the ~2.7us ACT_TABLE_LOAD before the FIRST
            # ACTIVATE; without this it lands on round 0's first exp, which
            # the PE (started early by the DMA ordering below) then waits
            # for. A dummy exp at t~0 pulls the load into the input-DMA
            # window.
            actw = rp.tile([128, 512], F32, tag="R", name="actwarm")
            nc.vector.memset(actw[0:1, 0:8], 1.0)
            nc.scalar.activation(actw[0:1, 0:8], actw[0:1, 0:8], Exp)

            # ---- input DMA ----
            if OPT_DMA:
                # Ordered by first use: xT split by 512-col windows (so
                # linqk(0)'s first pieces start ~7us before the last xT
                # bytes land), then just the m=0 / m=6 wqk column slices
                # (what linqk(0)/linqk(6) consume), then wv for linv,
                # deferring the remaining wqk columns (fillers, used from
                # round 1) and pw (used last).
                # NOTE: splitting xT by 512-col windows measured WORSE: the
                # DMA-completion semaphore waits coarsen to "#64 on this
                # ring" and the first linqk waited ~25us instead of ~19.
                # Each DMA dispatch costs ~0.6us on its engine queue, so the
                # ~25 input dispatches alternate between the sync and scalar
                # HW queues (ACT is idle at startup) and the two wqk m-slice
                # blocks ride one 3D-AP DMA each.
                def inq(i):
                    return nc.sync if i % 2 == 0 else nc.scalar

                # small wqk m0/m6 slices FIRST (land ~10.6us), then xT split
                # at the batch boundary: linqk(0/6)'s first two windows only
                # need the batch-0 half -> first matmul ~15us instead of
                # ~19us (the m-slices previously landed last, ~19us, because
                # they were dispatched after all 3.15MB of xT)
                for k in range(6):
                    sl = slice(k * 128, (k + 1) * 128)
                    inq(k).dma_start(out=two_blocks(wqk[k][:, :], 0, 128, 768),
                                     in_=two_blocks(wqk_e[sl, :], 0, 128, 768))
                for k in range(6):
                    sl = slice(k * 128, (k + 1) * 128)
                    inq(k).dma_start(out=xT[k][:, 0:N], in_=xT_e[sl, 0:N])
                for k in range(6):
                    sl = slice(k * 128, (k + 1) * 128)
                    inq(k).dma_start(out=xT[k][:, N:TOK], in_=xT_e[sl, N:TOK])
                for k in range(6):
                    sl = slice(k * 128, (k + 1) * 128)
                    inq(k).dma_start(out=wv[k], in_=wv_e[sl, :])
                nc.sync.dma_start(out=pb, in_=bcast_rows(pb_e[None, :], 128))
                for k in range(6):
                    sl = slice(k * 128, (k + 1) * 128)
                    inq(k).dma_start(out=two_blocks(wqk[k][:, :], 128, 640, 896),
                                     in_=two_blocks(wqk_e[sl, :], 128, 640, 896))
                for k in range(6):
                    sl = slice(k * 128, (k + 1) * 128)
                    inq(k).dma_start(out=pw[k], in_=pw_e[sl, :])
            else:
                for k in range(6):
                    sl = slice(k * 128, (k + 1) * 128)
                    nc.sync.dma_start(out=xT[k], in_=xT_e[sl, :])
                    nc.sync.dma_start(out=wqk[k], in_=wqk_e[sl, :])
                for k in range(6):
                    sl = slice(k * 128, (k + 1) * 128)
                    nc.sync.dma_start(out=wv[k], in_=wv_e[sl, :])
                nc.sync.dma_start(out=pb, in_=bcast_rows(pb_e[None, :], 128))
                for k in range(6):
                    sl = slice(k * 128, (k + 1) * 128)
                    nc.sync.dma_start(out=pw[k], in_=pw_e[sl, :])

            # ---- phase helpers (emission order = scheduling priority) ----
            def emit_linqk_piece(m, w0):
                if True:
                    wn = min(512, TOK - w0)
                    ps = ps_lin.tile([128, 512], F32, tag="lin", name=f"psqk{m}_{w0}")
                    for k in range(6):
                        nc.tensor.matmul(
                            ps[:, :wn],
                            lhsT=wqk[k][:, m * 128:(m + 1) * 128],
                            rhs=xT[k][:, w0:w0 + wn],
                            start=(k == 0), stop=(k == 5),
                        )
                    nc.vector.tensor_copy(qk[m][:, w0:w0 + wn], ps[:, :wn])

            def emit_linqk(m):
                for w0 in range(0, TOK, 512):
                    emit_linqk_piece(m, w0)

            def emit_linv_piece(b, j):
                t0, tn = TCH[j]
                if True:
                    vt = vaug[b][j]
                    for e0, en in [(0, 512), (512, 256)]:
                        ps = ps_lin.tile([128, 512], F32, tag="lin", name=f"psv{b}_{j}_{e0}")
                        for k in range(6):
                            nc.tensor.matmul(
                                ps[:tn, :en],
                                lhsT=xT[k][:, b * N + t0: b * N + t0 + tn],
                                rhs=wv[k][:, e0:e0 + en],
                                start=(k == 0), stop=(k == 5),
                            )
                        nh = en // HD
                        h0 = e0 // HD
                        dst = vt[:tn].rearrange("p (h s) -> p h s", s=65)[:, h0:h0 + nh, 0:HD]
                        src = ps[:tn, :en].rearrange("p (h s) -> p h s", s=HD)
                        nc.vector.tensor_copy(dst, src)
                    ones = vt[:tn].rearrange("p (h s) -> p h s", s=65)[:, :, HD:65]
                    nc.vector.memset(ones, 1.0)

            def emit_linv(b):
                for j in range(9):
                    emit_linv_piece(b, j)

            # ---- attention emission (per batch, per head pair) ----
            smt_all, smdd_all, site_row_all = {}, {}, {}

            def attn_setup(b):
                smt = [smtp.tile([128, 1056], F32, tag="smt", name=f"smt{b}_{t}")
                       for t in range(3)]
                for t in range(3):
                    nc.vector.memset(smt[t], 1.0)
                smt_all[b] = smt
                smdd_all[b] = dr.tile([4 * 3, 1056], F32, tag="smdd", name=f"smdd{b}")

            FILLER = []

            def drain(k):
                for _ in range(min(k, len(FILLER))):
                    FILLER.pop(0)()

            def emit_attn(b, hp):
                smt = smt_all[b]
                smdd = smdd_all[b]

                def site_row(hp2, qi):
                    sid = hp2 * 2 + qi
                    return smt[sid // 4], 32 * (sid % 4)

                def norm_site(hp2, qi, mul_eng, mul_eng2=None, dq=None):
                    q0, qn = QW[qi]
                    sid = hp2 * 2 + qi
                    drow = 4 * (sid // 4) + (sid % 4)
                    R = rp.tile([128, 512], F32, tag="R", name=f"R{b}_{hp2}_{qi}")
                    nc.sync.dma_start(
                        out=R[0:64, :qn],
                        in_=bcast_rows(smdd[drow:drow + 1, 0:qn], 64))
                    (dq or nc.sync).dma_start(
                        out=R[64:128, :qn],
                        in_=bcast_rows(smdd[drow:drow + 1, 512:512 + qn], 64))
                    qsl_l = slice(q0, q0 + qn)
                    mul_eng.tensor_mul(xstdT[b][hp2][0:64, qsl_l],
                                       xstdT[b][hp2][0:64, qsl_l], R[0:64, :qn])
                    (mul_eng2 or mul_eng).tensor_mul(
                        xstdT[b][hp2][64:128, qsl_l],
                        xstdT[b][hp2][64:128, qsl_l], R[64:128, :qn])

                def norm_cls(hp2, mul_eng, dq=None):
                    sid = hp2 * 2
                    drow = 4 * (sid // 4) + (sid % 4)
                    Rc = rp.tile([128, 512], F32, tag="R", name=f"Rc{b}_{hp2}")
                    nc.sync.dma_start(
                        out=Rc[0:64, 0:1],
                        in_=bcast_rows(smdd[drow:drow + 1, 1024:1025], 64))
                    (dq or nc.sync).dma_start(
                        out=Rc[64:128, 0:1],
                        in_=bcast_rows(smdd[drow:drow + 1, 1025:1026], 64))
                    mul_eng.tensor_mul(xstdT[b][hp2][0:64, 1024:1025],
                                       xstdT[b][hp2][0:64, 1024:1025], Rc[0:64, 0:1])
                    mul_eng.tensor_mul(xstdT[b][hp2][64:128, 1024:1025],
                                       xstdT[b][hp2][64:128, 1024:1025], Rc[64:128, 0:1])

                qt = qk[hp]
                kt = qk[6 + hp]
                if True:
                    for q0, qn in QW:
                        psO_a = ps_o.tile([65, 512], F32, tag="psO", name=f"psOa{b}_{hp}_{q0}")
                        psO_b = ps_o.tile([65, 512], F32, tag="psO", name=f"psOb{b}_{hp}_{q0}")
                        for kc, (t0, tn) in enumerate(TCH):
                            ksl = slice(b * N + t0, b * N + t0 + tn)
                            qsl = slice(b * N + q0, b * N + q0 + qn)
                            psS = ps_s.tile([128, 1024], F32, tag="psS", name=f"psS{b}_{hp}_{q0}_{kc}")
                            # two heads row-tiled concurrently (K=64 each)
                            nc.tensor.matmul(psS[:tn, 0:qn], lhsT=kt[0:64, ksl],
                                             rhs=qt[0:64, qsl], start=True, stop=True)
                            nc.tensor.matmul(psS[:tn, 512:512 + qn], lhsT=kt[64:128, ksl],
                                             rhs=qt[64:128, qsl], start=True, stop=True)
                            pt = ptp.tile([128, 1024], BF16, tag="pt", name=f"pt{b}_{hp}_{q0}_{kc}")
                            nc.scalar.activation(pt[:tn], psS[:tn], Exp, scale=SCALE)
                            first, last = (kc == 0), (kc == 8)
                            nc.tensor.matmul(psO_a[:, :qn],
                                             lhsT=vaug[b][kc][:tn, 2 * hp * 65:2 * hp * 65 + 65],
                                             rhs=pt[:tn, 0:qn], start=first, stop=last)
                            nc.tensor.matmul(psO_b[:, :qn],
                                             lhsT=vaug[b][kc][:tn, (2 * hp + 1) * 65:(2 * hp + 1) * 65 + 65],
                                             rhs=pt[:tn, 512:512 + qn], start=first, stop=last)
                        # stash sums into the batch tile and the UNNORMALIZED
                        # O^T into xstdT (bf16); normalize in place per 2 hp.
                        st, row = site_row(hp, q0 // 512)
                        nc.vector.tensor_copy(st[row:row + 1, 0:qn], psO_a[64:65, :qn])
                        nc.vector.tensor_copy(st[row:row + 1, 512:512 + qn], psO_b[64:65, :qn])
                        qsl_l = slice(q0, q0 + qn)
                        nc.vector.tensor_copy(xstdT[b][hp][0:64, qsl_l], psO_a[0:64, :qn])
                        nc.vector.tensor_copy(xstdT[b][hp][64:128, qsl_l], psO_b[0:64, :qn])
                        # extra drains in the last head pair: the 9 proj(0)
                        # fillers must overlap attn(1,5), not pile into the tail
                        # (reserving pieces for the tail measured ~14us slower)
                        drain(4 if hp == HP - 1 else 1)

                    # ---- last query token (qtok = N-1) for this head pair ----
                    psc = ps_s.tile([128, 18], F32, tag="psS", name=f"psc{b}_{hp}")
                    nc.vector.memset(psc, 0.0)
                    # NOTE: keep hh-major. kc-major (interleaving the two
                    # row-tile bases back-to-back into the SAME single-bank
                    # psum) HANGS the device -- unlike the main S pairs,
                    # whose concurrent halves write different PSUM banks.
                    for hh in range(2):
                        hsl = slice(hh * 64, hh * 64 + 64)
                        for kc, (t0, tn) in enumerate(TCH):
                            nc.tensor.matmul(
                                psc[:tn, hh * 9 + kc: hh * 9 + kc + 1],
                                lhsT=kt[hsl, b * N + t0: b * N + t0 + tn],
                                rhs=qt[hsl, b * N + 1024: b * N + 1025],
                                start=True, stop=True,
                            )
                    ptc = ptp.tile([128, 18], BF16, tag="pt", name=f"ptc{b}_{hp}")
                    nc.scalar.activation(ptc, psc, Exp, scale=SCALE)
                    psOc_a = ps_o.tile([65, 512], F32, tag="psO", name=f"psOca{b}_{hp}")
                    psOc_b = ps_o.tile([65, 512], F32, tag="psO", name=f"psOcb{b}_{hp}")
                    for hh, psOc in ((0, psOc_a), (1, psOc_b)):
                        h = 2 * hp + hh
                        for kc, (t0, tn) in enumerate(TCH):
                            nc.tensor.matmul(
                                psOc[:, 0:1],
                                lhsT=vaug[b][kc][:tn, h * 65: h * 65 + 65],
                                rhs=ptc[:tn, hh * 9 + kc: hh * 9 + kc + 1],
                                start=(kc == 0), stop=(kc == 8),
                            )
                    st, row = site_row(hp, 0)
                    nc.vector.tensor_copy(st[row:row + 1, 1024:1025], psOc_a[64:65, 0:1])
                    nc.vector.tensor_copy(st[row:row + 1, 1025:1026], psOc_b[64:65, 0:1])
                    nc.vector.tensor_copy(xstdT[b][hp][0:64, 1024:1025], psOc_a[0:64, 0:1])
                    nc.vector.tensor_copy(xstdT[b][hp][64:128, 1024:1025], psOc_b[0:64, 0:1])
                    drain(1)

                    # ---- normalization for this smt tile (every 2nd hp) ----
                    if hp % 2 == 1:
                        t = hp // 2
                        last = hp == HP - 1
                        # 1/s = exp(-ln s) on ACT (2 x ~1.2us). The 6.7us DVE
                        # reciprocal used mid-kernel previously sat in the
                        # in-order DVE stream exactly when psO/filler psum
                        # drains queue behind it -> PE starves and the HAM
                        # drops the PE clock to 1.2GHz. The single loaded
                        # table set holds both Exp and Ln (no switch cost),
                        # and ACT has a natural dip at round boundaries.
                        if OPT_LNEXP or (last and b == 1):
                            nc.scalar.activation(smt[t][0:97, :], smt[t][0:97, :],
                                                 mybir.ActivationFunctionType.Ln)
                            nc.scalar.activation(smt[t][0:97, :], smt[t][0:97, :],
                                                 Exp, scale=-1.0)
                        else:
                            nc.vector.reciprocal(smt[t][0:97, :], smt[t][0:97, :])
                        nc.sync.dma_start(
                            out=smdd[4 * t:4 * t + 4, :],
                            in_=bass.AP(tensor=smt[t].tensor, offset=smt[t].offset,
                                        ap=[[32 * smt[t].ap[0][0], 4]] + list(smt[t].ap[1:])),
                        )
                        if last:
                            # tail: the finals wait on ALL norm muls -> load-
                            # balance the 8 big [64,512] muls across GpSimd
                            # (~0.83us/op) and DVE (~0.4us/op): 3 on GpSimd,
                            # 5 on DVE finishes ~1us sooner than an even or
                            # per-chunk split. cls tinies ride GpSimd. For
                            # b==1 the R-broadcast dispatches (each ~0.6us of
                            # engine-queue time) ride the otherwise-idle
                            # scalar HW queue instead of serializing on sync
                            # behind the output DMAs.
                            dq = nc.scalar if b == 1 else None
                            seq = [nc.gpsimd, nc.vector, nc.vector,
                                   nc.gpsimd, nc.vector, nc.vector,
                                   nc.gpsimd, nc.vector]
                            si_ = 0
                            for hp2 in (hp - 1, hp):
                                for qi in range(2):
                                    norm_site(hp2, qi, seq[si_], seq[si_ + 1], dq=dq)
                                    si_ += 2
                                norm_cls(hp2, nc.gpsimd, dq=dq)
                        else:
                            # mid-kernel both chunks ride GpSimd (DVE busy
                            # with drains)
                            for hp2 in (hp - 1, hp):
                                for qi in range(2):
                                    norm_site(hp2, qi, nc.gpsimd)
                                norm_cls(hp2, nc.gpsimd)

            PDONE = set()

            def emit_proj_piece(b, j):
                if (b, j) in PDONE:
                    return
                PDONE.add((b, j))
                t0, tn = TCH[j]
                if True:
                    ot = outp.tile([128, C], BF16, tag="ot", name=f"ot{b}_{j}")
                    for e0, en in [(0, 512), (512, 256)]:
                        ps = ps_lin.tile([128, 512], F32, tag="lin", name=f"psp{b}_{j}_{e0}")
                        for k in range(6):
                            nc.tensor.matmul(
                                ps[:tn, :en],
                                lhsT=xstdT[b][k][:, t0:t0 + tn],
                                rhs=pw[k][:, e0:e0 + en],
                                start=(k == 0), stop=(k == 5),
                            )
                        nc.vector.tensor_add(ot[:tn, e0:e0 + en], ps[:tn, :en], pb[:tn, e0:e0 + en])
                    nc.sync.dma_start(
                        out=out_e[b * N + t0: b * N + t0 + tn, :],
                        in_=ot[:tn, :],
                    )

            def emit_proj(b):
                for j in range(9):
                    emit_proj_piece(b, j)

            # ---- split-K proj for batch 1 (tail shortening) ----
            # proj(1,j) = sum_k xstdT[1][k].T @ pw[k]; the k=0..3 partial has
            # no dependency on the FINAL norm round (hp 4/5), so it runs as
            # round-4/5 filler, parked bf16 in dead qk slots (qk[m] for early
            # head pairs is last read in round <=3). The serial tail is then
            # only k=4..5 + a combine add. bf16 partials add ~3e-3 rel err
            # (gate is 2e-2).
            PART_SLOT = [(0, 0), (0, 1025), (1, 0), (1, 1025), (2, 0),
                         (2, 1025), (3, 0), (3, 1025), (6, 0)]

            def emit_proj1_partial(j):
                t0, tn = TCH[j]
                qi_, off = PART_SLOT[j]
                part = qk[qi_][:, off:off + C]
                for e0, en in [(0, 512), (512, 256)]:
                    ps = ps_lin.tile([128, 512], F32, tag="lin", name=f"pspa1_{j}_{e0}")
                    for k in range(4):
                        nc.tensor.matmul(
                            ps[:tn, :en],
                            lhsT=xstdT[1][k][:, t0:t0 + tn],
                            rhs=pw[k][:, e0:e0 + en],
                            start=(k == 0), stop=(k == 3),
                        )
                    # fold the bias into the partial here
                    nc.vector.tensor_add(part[:tn, e0:e0 + en], ps[:tn, :en],
                                         pb[:tn, e0:e0 + en])

            def emit_proj1_final(j):
                # attention is done by now: the ps_s pool (2x 2-bank bufs) is
                # free -- both e-chunks of a piece share ONE [128,1024] psum
                # (different banks), so there is a single combine-add per
                # piece and two pieces pipeline through the pool. Keeping
                # these off ps_lin matters: the junk screen parks a buf there.
                t0, tn = TCH[j]
                qi_, off = PART_SLOT[j]
                part = qk[qi_][:, off:off + C]
                ot = outp.tile([128, C], BF16, tag="ot", name=f"ot1_{j}")
                ps = ps_s.tile([128, 1024], F32, tag="psS", name=f"pspb1_{j}")
                for e0, en in [(0, 512), (512, 256)]:
                    for k in range(4, 6):
                        nc.tensor.matmul(
                            ps[:tn, e0:e0 + en],
                            lhsT=xstdT[1][k][:, t0:t0 + tn],
                            rhs=pw[k][:, e0:e0 + en],
                            start=(k == 4), stop=(k == 5),
                        )
                nc.vector.tensor_add(ot[:tn, 0:C], ps[:tn, 0:C], part[:tn, 0:C])
                # alternate HW queues: ACT is idle in the tail and each
                # dispatch costs ~0.6us of serial engine-queue time
                (nc.scalar if j % 2 == 0 else nc.sync).dma_start(
                    out=out_e[N + t0: N + t0 + tn, :],
                    in_=ot[:tn, :],
                )

            # ---- interleaved emission schedule ----
            # Emission order ~= static schedule priority. Attention leads;
            # LIN/proj pieces are queued as fillers drained between attention
            # stages (so they fill PE idle instead of blocking attention).
            attn_setup(0)
            attn_setup(1)
            with nc.named_scope("lin_head"):
                # window order matches DMA landing order: the first two
                # 512-windows of each head chunk need only the batch-0 xT
                # half; the rest lands while they run
                for m in (0, 6):
                    emit_linqk_piece(m, 0)
                    emit_linqk_piece(m, 512)
                for m in (0, 6):
                    for w0 in (1024, 1536, 2048):
                        emit_linqk_piece(m, w0)
                emit_linv(0)
                emit_linv(1)
            for hp in range(1, HP):
                FILLER.extend([
                    (lambda m=hp, w=w0: emit_linqk_piece(m, w))
                    for w0 in range(0, TOK, 512)
                ] + [
                    (lambda m=6 + hp, w=w0: emit_linqk_piece(m, w))
                    for w0 in range(0, TOK, 512)
                ])
            # round-boundary screens measured neutral-to-slightly-worse
            # (+1.4us): the boundary stalls are ACT-pipeline refill, not
            # idle-window throttle. Off by default.
            NJUNKR = int(os.environ.get("KJUNKR", "0"))

            def junk_screen(n, nm):
                # dependency-free wide matmuls emitted where the PE would
                # otherwise sit idle long enough for the HAM to halve the
                # clock; results are never read.
                jp = ps_lin.tile([128, 512], F32, tag="lin", name=nm)
                for i in range(n):
                    nc.tensor.matmul(jp[:, 0:512], lhsT=pw[0][:, 0:128],
                                     rhs=xT[0][:, 0:512],
                                     start=(i == 0), stop=(i == n - 1))

            emit_attn(0, 0)
            emit_attn(1, 0)
            for hp in range(1, HP):
                if NJUNKR:
                    junk_screen(NJUNKR, f"jnkr{hp}")
                while FILLER and len(FILLER) > 10 * (HP - 1 - hp):
                    FILLER.pop(0)()
                emit_attn(0, hp)
                if hp == HP - 2:
                    # proj(1) k=0..3 partials: ready (xstdT[1][0..3] was
                    # normalized after round 3) -> fill rounds 4-5
                    FILLER.extend([(lambda j=j: emit_proj1_partial(j))
                                   for j in range(9)])
                if hp == HP - 1:
                    FILLER.extend([(lambda j=j: emit_proj_piece(0, j))
                                   for j in range(9)])
                emit_attn(1, hp)
            with nc.named_scope("proj_tail"):
                while FILLER:
                    FILLER.pop(0)()
                # warm-keepers: the PE would otherwise idle ~6-8us through
                # the final norm chain (ln/exp + DRAM roundtrip + muls) and
                # the HAM would halve the clock right before the k=4..5
                # finals; a screen of dependency-free wide matmuls holds
                # activity up. Results are never read.
                NJUNK = int(os.environ.get("KJUNK", "40"))
                if NJUNK:
                    jps = ps_o.tile([65, 512], F32, tag="psO", name="jnk")
                    for i in range(NJUNK):
                        nc.tensor.matmul(
                            jps[:, 0:512],
                            lhsT=pw[0][:, 0:65],
                            rhs=xT[0][:, 0:512],
                            start=(i == 0), stop=(i == NJUNK - 1),
                        )
                for j in range(9):
                    emit_proj1_final(j)
    return nc


def _fuse_ldweights(nc):
    """Tile splits every matmul into standalone LDWEIGHTS + MATMUL; with
    this walrus build (--enable-ldw-opt=false) the pair executes serially,
    exposing ~100ns of weight-load per matmul. Re-fuse: drop the standalone
    LDW and let the matmul self-load (ldweights=True), moving any waits /
    sem updates onto the matmul (funnel pass then enforces the 1-wait cap)."""
    for f in nc.m.functions:
        for blk in f.blocks:
            insts = blk.instructions
            new = []
            pending = []  # waits/updates from deleted LDWs awaiting next MM
            changed = False
            for inst in insts:
                tn = type(inst).__name__
                if tn == "InstLdweights":
                    si = inst.sync_info
                    if si is not None and (si.on_wait or si.on_update):
                        pending.append((list(si.on_wait), list(si.on_update)))
                    changed = True
                    continue
                if tn == "InstMatmult":
                    inst.ldweights = True
                    if pending:
                        si = inst.sync_info
                        if si is None:
                            inst.sync_info = mybir.SyncInfo(on_wait=[], on_update=[])
                            si = inst.sync_info
                        w = list(si.on_wait)
                        u = list(si.on_update)
                        for pw_, pu_ in pending:
                            w.extend(pw_)
                            u.extend(pu_)
                        si.on_wait = w
                        si.on_update = u
                        pending = []
                new.append(inst)
            assert not pending, "dangling LDW sync with no following matmul"
            if changed:
                blk.instructions = new


def _funnel_pe_waits(nc):
    """Walrus allows only one sync-wait slot per engine instruction.

    Semaphores are monotonic and each engine's sequencer executes its
    stream in order, so a wait already executed by an earlier same-engine
    instruction is redundant later. Strip covered waits; if an engine
    instruction still needs >=2 waits, hoist them onto inserted
    single-wait NoOps directly before it (the sequencer executes those
    first). DMA copies / drains / event-sems use different sync hardware
    and are left untouched.
    """
    SKIP = {"InstEventSemaphore", "InstNoOp",
            "InstIncSwdgeSem", "InstTensorLoad", "InstTensorSave"}
    for f in nc.m.functions:
        for blk in f.blocks:
            insts = blk.instructions
            new = []
            seen = {e: {} for e in mybir.ALL_ENGINES}
            changed = False
            for inst in insts:
                si = getattr(inst, "sync_info", None)
                eng = inst.engine
                tn = type(inst).__name__
                if (eng in seen and tn not in SKIP
                        and si is not None and si.on_wait):
                    sn = seen[eng]
                    waits = [w for w in si.on_wait
                             if not (w.wait_mode == "sem-ge-imm"
                                     and sn.get(w.id, -1) >= w.wait_value)]
                    if tn != "InstDMACopy":
                        # DMA waits execute ring-side, not on the sequencer:
                        # they don't advance the engine's observed state
                        for w in waits:
                            if w.wait_mode == "sem-ge-imm":
                                sn[w.id] = max(sn.get(w.id, -1), w.wait_value)
                    if len(waits) > 1:
                        for wi, w in enumerate(waits):
                            noop = mybir.InstNoOp(
                                name=f"{inst.name}_wfun{wi}",
                                sync_info=mybir.SyncInfo(on_wait=[w], on_update=[]),
                                bass_nofuse=True,
                                text_hint="wait_funnel",
                            )
                            noop.engine = eng
                            new.append(noop)
                            if w.wait_mode == "sem-ge-imm":
                                sn[w.id] = max(sn.get(w.id, -1), w.wait_value)
                        waits = []
                    if len(waits) != len(si.on_wait):
                        si.on_wait = waits
                        changed = True
                new.append(inst)
            if changed or len(new) != len(insts):
                blk.instructions = new


_NC_CACHE = None


def get_nc():
    global _NC_CACHE
    if _NC_CACHE is None:
        _NC_CACHE = build_nc()
    return _NC_CACHE


def make_in_maps(x, qkv_w, proj_w, proj_b):
    bf = ml_dtypes.bfloat16
    wqkT = np.ascontiguousarray(np.asarray(qkv_w, np.float32)[:DQK].T).astype(bf)
    wvT = np.ascontiguousarray(np.asarray(qkv_w, np.float32)[DQK:].T).astype(bf)
    pwT = np.ascontiguousarray(np.asarray(proj_w, np.float32).T).astype(bf)
    pb = np.asarray(proj_b, np.float32)
    x = np.asarray(x, np.float32)
    in_maps = []
    for i in range(NCORES):
        xs = x[NB * i: NB * (i + 1)].reshape(TOK, C)
        xT = np.ascontiguousarray(xs.T).astype(bf)
        in_maps.append({"xT": xT, "wqkT": wqkT, "wvT": wvT, "pwT": pwT, "pb": pb})
    return in_maps


def _ensure_ntff_hook():
    """The agent image's antenv lacks axon_hooks; shim it so trace=True
    (profiling-only path) works instead of crashing on import."""
    import sys
    import types

    try:
        import antenv.axon_hooks  # noqa: F401
        return
    except ImportError:
        pass
    mod = types.ModuleType("antenv.axon_hooks")
    state = {"h": None}
    mod.set_axon_ntff_profile_hook = lambda h: state.__setitem__("h", h)
    mod.get_axon_ntff_profile_hook = lambda: state["h"]
    sys.modules["antenv.axon_hooks"] = mod
    import antenv

    antenv.axon_hooks = mod
    from trn_agent_boot.trn_boot import _ntff_profile_via_ctypes

    mod.set_axon_ntff_profile_hook(
        _ntff_profile_via_ctypes("/opt/axon/libaxon_pjrt.so")
    )


def kernel(x, qkv_w, proj_w, proj_b, H=None, W=None, _trace=False):
    from concourse.bass_utils import run_bass_kernel_spmd

    if _trace:
        _ensure_ntff_hook()
    nc = get_nc()
    if not getattr(nc, "_pe_waits_funneled", False):
        import os as _os
        if _os.environ.get("KFUSE_LDW", "1") == "1":
            _fuse_ldweights(nc)
        _funnel_pe_waits(nc)
        nc._pe_waits_funneled = True
    in_maps = make_in_maps(x, qkv_w, proj_w, proj_b)
    res = run_bass_kernel_spmd(nc, in_maps, core_ids=list(range(NCORES)), trace=_trace)
    out = np.concatenate(
        [r["out"].reshape(NB, N, C) for r in res.results], axis=0
    ).astype(np.float32)
    if _trace:
        kernel.last_exec_time_ns = res.exec_time_ns
        kernel.last_results = res
    return out



# revision 42
# speedup vs baseline: 1.1895x; 1.1895x over previous
"""Multi-head attention (ViT-style, N=1025 tokens incl. cls) on 8 TRN2 NeuronCores.

Reference semantics: the "separate cls-token attention" branch of the reference
is mathematically identical to row 0 of standard attention (same logits, same
softmax, same values), so the output is exactly
    out = softmax(Q K^T * hd^-0.5) V -> proj -> + bias.

Sharding: data-parallel over batch: B=16 -> 2 batches per core, weights
replicated, no collectives. ~422us HW exec on silicon (run variance
~+/-5us), rel err ~3.3e-3.

Changes vs the ~470us predecessor:
  - normalization multiplies ride the (otherwise idle) GpSimd engine
    mid-kernel, so the in-order DVE stream never blocks lin-filler psum
    drains (those stalls reset the PE DVFS p-state: a blocked PE restarts
    at 1.2GHz and needs ~3us of continuous work to regain 2.4GHz)
  - ALL normalizations compute 1/s as exp(-ln s) on ACT (2 x ~1.2us; the
    one loaded table set holds both Exp and Ln) -- the 6.7us DVE
    reciprocal sat in the in-order DVE stream exactly where psO/filler
    psum drains queue behind it, starving the PE into HAM clock drops
  - split-K proj for batch 1: the k=0..3 partial (independent of the
    final norm round) runs as round-4/5 filler, partials parked bf16 in
    dead qk slots; the serial tail is only k=4..5 + one combine-add per
    piece (phase-B psums ride the by-then-free ps_s pool: both e-chunks
    in one [128,1024] tile, one DVE add per piece)
  - input DMAs ordered by first use (the small m=0/m=6 wqk slices FIRST,
    then xT split at the batch boundary so linqk's first windows start at
    ~14us instead of ~19us; wv next, rest deferred), with dispatches
    alternating sync/scalar HW queues (each dispatch is ~0.6us of serial
    engine-queue time) and the two wqk m-slice blocks merged into one
    3D-AP DMA; lin_head emission follows DMA landing order
  - a dummy exp at t~0 pulls the ~2.7us ACT_TABLE_LOAD into the
    input-DMA window (the early-started PE otherwise waits for it at
    round 0's first exp)
  - tail R-broadcast + output-DMA dispatches alternate sync/scalar HW
    queues; the final norm's 8 big muls are load-balanced 3:5 over
    GpSimd (~0.83us/op) vs DVE (~0.4us/op)
  - a ~5us screen of dependency-free matmuls covers the tail norm chain
    so the HAM doesn't halve the PE clock right before the k=4..5 finals
  - proj(0) fillers drain 4-per-window during the last head pair instead of
    piling into the serial tail
  - per-(b,j) proj outputs accumulate into one [128,768] tile and ship in a
    single DMA (half the output-DMA dispatch cost on the sync engine)
Failed experiments (measured SLOWER, do not retry):
  - cls-pass S matmuls kc-major (pairing the two row-tile bases into the
    same single-bank psum): DEVICE HANG -- concurrent row-tile halves
    must target different PSUM banks (main S pairs do)
  - xT DMAs split by 512-col windows: completion-sem waits coarsen to
    "#64 on this ring", first linqk waited ~25us instead of ~19
  - round-boundary junk screens (KJUNKR=3) + longer tail screen
    (KJUNK=44): +1.4us
  - fp8 DoubleRow S-matmuls: on silicon a DR matmul streams 2F moving
    elements (fold dim rides the moving stream), costing the same as the two
    bf16 row-tiled matmuls it replaces -- plus cast/fold overhead
  - balanced 9x~114 token chunks: stationary operands lose 256B alignment
    -> LDWEIGHTS penalty, ~+90us
  - software-pipelining O one chunk behind S, and JIT V-piece emission:
    both shuffle work into the DVE stream at points that block psum drains
  - reciprocal_approx_fast / gpsimd-issued DMAs: unsupported by this
    walrus/runtime (ISA-wrong-length / device hang)
  - reserving proj(0) fillers for the tail norm-chain window: +14us (x2)
  - splitting the mid-kernel DVE reciprocal into halves: +13us solo (the
    smdd DMA needs both halves, so the chain got longer, not shorter)
  - rp pool 5-6 bufs: SBUF overflow (no per-partition headroom left)
  - PE warmup junk matmuls during the input-DMA wait: neutral within noise
  - xT input DMAs on the scalar-engine queue: slightly slower

Per-core layout strategy (matmul operands bf16, f32 PSUM accumulation):
  - Host pre-transposes x / weights so contraction dims land on partitions.
  - qkT = wqkT.T @ xT      -> [1536, tok]  (Q^T,K^T: head dim on partitions)
  - V   = xT.T @ wvT       -> [tok, 768] in 65-stride head layout with a
    ones column per head (softmax sums ride the O matmul for free)
  - S^T = K_h^T.T @ Q_h^T  -> [ktok, qtok], two heads row-tiled concurrently
    (tile_position from base partitions 0/64); query windows 2x512, the last
    query column batched per head pair into a [128, 18] collector
  - P^T = exp(S^T * scale) on ScalarE, one [128, 1024] instr per k-chunk
    (ACT costs (N+352) cycles -> wide instrs; no max-subtraction needed
    since |logits| < ~4 for this distribution)
  - O^T = Vaug_h.T @ P^T   -> [65, qtok] PSUM; row 64 = softmax sums
  - unnormalized O^T is cast straight into xstdT (bf16); sums are collected
    into partition-aligned batch tiles (rows 0/32/64/96), one wide
    reciprocal per 4 sites (DVE mid-kernel, ACT exp(-ln) at the tail),
    DRAM-roundtrip partition-broadcast, then in-place multiply on GpSimd
    (no engine can broadcast across partitions; DMA can, from DRAM)
  - y = xstdT.T @ pwT + bias -> [tok, 768] -> bf16 out DMA (host casts f32)

Emission order doubles as the static-schedule priority (Tile list-scheduler):
attention leads, LIN-QK/LIN-V/proj pieces are queued as fillers drained
between attention stages so they soak up PE idle during the ACT-paced
attention pipeline.

Post-scheduling passes (this walrus allows ONE sync wait per engine
instruction): standalone LDWEIGHTS are re-fused into matmuls, then excess
waits are hoisted onto single-wait PE NoOps (semaphores are monotonic and
each sequencer executes in order, so earlier-covered waits are dropped).
"""

import os

import numpy as np
import ml_dtypes

import concourse.bass as bass
import concourse.mybir as mybir
import concourse.tile as tile

# optimization gates (bisect switches)
OPT_DMA = os.environ.get("KOPT_DMA", "1") == "1"
OPT_LNEXP = os.environ.get("KOPT_LNEXP", "1") == "1"

NCORES = 8
B, N, C = 16, 1025, 768
NB = B // NCORES          # batches per core
H = 12                    # heads
HD = C // H               # 64
HP = H // 2               # head pairs
TOK = NB * N              # tokens per core (2050)
SCALE = float(HD) ** -0.5
DQK = 2 * C               # 1536
F32 = mybir.dt.float32
BF16 = mybir.dt.bfloat16
Exp = mybir.ActivationFunctionType.Exp

# per-batch token chunks (attention / V / proj tiling): 8 x 128 + 1.
# NOTE: keep 128-chunks — a balanced 9 x ~114 split measured ~90us SLOWER
# (stationary operand offsets lose 256B alignment -> LDWEIGHTS penalty)
TCH = [(j * 128, 128) for j in range(8)] + [(1024, 1)]
# query-token windows (PSUM bank = 512 f32); last column handled in batched pass
QW = [(0, 512), (512, 512)]


def bcast_rows(ap_row, nrows):
    """AP reading one [1, n] row replicated across nrows partitions."""
    return bass.AP(
        tensor=ap_row.tensor,
        offset=ap_row.offset,
        ap=[[0, nrows]] + list(ap_row.ap[1:]),
    )


def two_blocks(ap2d, c0, w, c1):
    """3D AP over two equal-width column blocks [c0:c0+w] and [c1:c1+w] of a
    [P, ...] 2D AP -- lets one DMA cover both wqk m-slices."""
    pdim, (cstride, _) = ap2d.ap[0], ap2d.ap[1]
    return bass.AP(
        tensor=ap2d.tensor,
        offset=ap2d.offset + c0 * cstride,
        ap=[list(pdim), [(c1 - c0) * cstride, 2], [cstride, w]],
    )


def build_nc():
    nc = bass.Bass()
    xT_e = nc.declare_dram_parameter("xT", [C, TOK], BF16, isOutput=False)
    wqk_e = nc.declare_dram_parameter("wqkT", [C, DQK], BF16, isOutput=False)
    wv_e = nc.declare_dram_parameter("wvT", [C, C], BF16, isOutput=False)
    pw_e = nc.declare_dram_parameter("pwT", [C, C], BF16, isOutput=False)
    pb_e = nc.declare_dram_parameter("pb", [C], F32, isOutput=False)
    out_e = nc.declare_dram_parameter("out", [TOK, C], BF16, isOutput=True)

    with tile.TileContext(nc) as tc:
        with (
            tc.tile_pool(name="big", bufs=1) as big,
            tc.tile_pool(name="ps_lin", bufs=2, space="PSUM") as ps_lin,
            tc.tile_pool(name="ps_s", bufs=2, space="PSUM") as ps_s,
            tc.tile_pool(name="ps_o", bufs=2, space="PSUM") as ps_o,
            tc.tile_pool(name="ptp", bufs=4) as ptp,
            tc.tile_pool(name="rp", bufs=3) as rp,
            tc.tile_pool(name="smtp", bufs=6) as smtp,
            tc.tile_pool(name="dr", bufs=6, space="DRAM") as dr,
            tc.tile_pool(name="outp", bufs=3) as outp,
        ):
            # ---- persistent SBUF tensors (static: one slot per tag) ----
            def big_tile(shape, dtype, nm):
                return big.tile(shape, dtype, tag=nm, name=nm)

            xT = [big_tile([128, TOK], BF16, f"xT{k}") for k in range(6)]


            wqk = [big_tile([128, DQK], BF16, f"wqk{k}") for k in range(6)]
            wv = [big_tile([128, C], BF16, f"wv{k}") for k in range(6)]
            pw = [big_tile([128, C], BF16, f"pw{k}") for k in range(6)]
            pb = big_tile([128, C], F32, "pb")
            # Q^T|K^T chunks: m 0..5 = Q (heads 2m,2m+1), 6..11 = K
            qk = [big_tile([128, TOK], BF16, f"qk{m}") for m in range(12)]
            # V with 65-stride head layout (col 64 of each head block = ones)
            vaug = [[big_tile([128, 65 * H], BF16, f"vaug{b}_{j}")
                     for j in range(9)] for b in range(NB)]
            # attention output transposed, per c-chunk (= head pair)
            xstdT = [[big_tile([128, N], BF16, f"xstdT{b}_{k}")
                      for k in range(6)] for b in range(NB)]
            # ---- ACT warmup ----
            # walrus inserts the ~2.7us ACT_TABLE_LOAD before the FIRST
            # ACTIVATE; without this it lands on round 0's first exp, which
            # the PE (started early by the DMA ordering below) then waits
            # for. A dummy exp at t~0 pulls the load into the input-DMA
            # window.
            actw = rp.tile([128, 512], F32, tag="R", name="actwarm")
            nc.vector.memset(actw[0:1, 0:8], 1.0)
            nc.scalar.activation(actw[0:1, 0:8], actw[0:1, 0:8], Exp)

            # ---- input DMA ----
            if OPT_DMA:
                # Ordered by first use: xT split by 512-col windows (so
                # linqk(0)'s first pieces start ~7us before the last xT
                # bytes land), then just the m=0 / m=6 wqk column slices
                # (what linqk(0)/linqk(6) consume), then wv for linv,
                # deferring the remaining wqk columns (fillers, used from
                # round 1) and pw (used last).
                # NOTE: splitting xT by 512-col windows measured WORSE: the
                # DMA-completion semaphore waits coarsen to "#64 on this
                # ring" and the first linqk waited ~25us instead of ~19.
                # Each DMA dispatch costs ~0.6us on its engine queue, so the
                # ~25 input dispatches alternate between the sync and scalar
                # HW queues (ACT is idle at startup) and the two wqk m-slice
                # blocks ride one 3D-AP DMA each.
                def inq(i):
                    return nc.sync if i % 2 == 0 else nc.scalar

                # small wqk m0/m6 slices FIRST (land ~10.6us), then xT split
                # at the batch boundary: linqk(0/6)'s first two windows only
                # need the batch-0 half -> first matmul ~15us instead of
                # ~19us (the m-slices previously landed last, ~19us, because
                # they were dispatched after all 3.15MB of xT)
                for k in range(6):
                    sl = slice(k * 128, (k + 1) * 128)
                    inq(k).dma_start(out=two_blocks(wqk[k][:, :], 0, 128, 768),
                                     in_=two_blocks(wqk_e[sl, :], 0, 128, 768))
                for k in range(6):
                    sl = slice(k * 128, (k + 1) * 128)
                    inq(k).dma_start(out=xT[k][:, 0:N], in_=xT_e[sl, 0:N])
                for k in range(6):
                    sl = slice(k * 128, (k + 1) * 128)
                    inq(k).dma_start(out=xT[k][:, N:TOK], in_=xT_e[sl, N:TOK])
                for k in range(6):
                    sl = slice(k * 128, (k + 1) * 128)
                    inq(k).dma_start(out=wv[k], in_=wv_e[sl, :])
                nc.sync.dma_start(out=pb, in_=bcast_rows(pb_e[None, :], 128))
                for k in range(6):
                    sl = slice(k * 128, (k + 1) * 128)
                    inq(k).dma_start(out=two_blocks(wqk[k][:, :], 128, 640, 896),
                                     in_=two_blocks(wqk_e[sl, :], 128, 640, 896))
                for k in range(6):
                    sl = slice(k * 128, (k + 1) * 128)
                    inq(k).dma_start(out=pw[k], in_=pw_e[sl, :])
            else:
                for k in range(6):
                    sl = slice(k * 128, (k + 1) * 128)
                    nc.sync.dma_start(out=xT[k], in_=xT_e[sl, :])
                    nc.sync.dma_start(out=wqk[k], in_=wqk_e[sl, :])
                for k in range(6):
                    sl = slice(k * 128, (k + 1) * 128)
                    nc.sync.dma_start(out=wv[k], in_=wv_e[sl, :])
                nc.sync.dma_start(out=pb, in_=bcast_rows(pb_e[None, :], 128))
                for k in range(6):
                    sl = slice(k * 128, (k + 1) * 128)
                    nc.sync.dma_start(out=pw[k], in_=pw_e[sl, :])

            # ---- phase helpers (emission order = scheduling priority) ----
            def emit_linqk_piece(m, w0):
                if True:
                    wn = min(512, TOK - w0)
                    ps = ps_lin.tile([128, 512], F32, tag="lin", name=f"psqk{m}_{w0}")
                    for k in range(6):
                        nc.tensor.matmul(
                            ps[:, :wn],
                            lhsT=wqk[k][:, m * 128:(m + 1) * 128],
                            rhs=xT[k][:, w0:w0 + wn],
                            start=(k == 0), stop=(k == 5),
                        )
                    nc.vector.tensor_copy(qk[m][:, w0:w0 + wn], ps[:, :wn])

            def emit_linqk(m):
                for w0 in range(0, TOK, 512):
                    emit_linqk_piece(m, w0)

            def emit_linv_piece(b, j):
                t0, tn = TCH[j]
                if True:
                    vt = vaug[b][j]
                    for e0, en in [(0, 512), (512, 256)]:
                        ps = ps_lin.tile([128, 512], F32, tag="lin", name=f"psv{b}_{j}_{e0}")
                        for k in range(6):
                            nc.tensor.matmul(
                                ps[:tn, :en],
                                lhsT=xT[k][:, b * N + t0: b * N + t0 + tn],
                                rhs=wv[k][:, e0:e0 + en],
                                start=(k == 0), stop=(k == 5),
                            )
                        nh = en // HD
                        h0 = e0 // HD
                        dst = vt[:tn].rearrange("p (h s) -> p h s", s=65)[:, h0:h0 + nh, 0:HD]
                        src = ps[:tn, :en].rearrange("p (h s) -> p h s", s=HD)
                        nc.vector.tensor_copy(dst, src)
                    ones = vt[:tn].rearrange("p (h s) -> p h s", s=65)[:, :, HD:65]
                    nc.vector.memset(ones, 1.0)

            def emit_linv(b):
                for j in range(9):
                    emit_linv_piece(b, j)

            # ---- attention emission (per batch, per head pair) ----
            smt_all, smdd_all, site_row_all = {}, {}, {}

            def attn_setup(b):
                smt = [smtp.tile([128, 1056], F32, tag="smt", name=f"smt{b}_{t}")
                       for t in range(3)]
                for t in range(3):
                    nc.vector.memset(smt[t], 1.0)
                smt_all[b] = smt
                smdd_all[b] = dr.tile([4 * 3, 1056], F32, tag="smdd", name=f"smdd{b}")

            FILLER = []

            def drain(k):
                for _ in range(min(k, len(FILLER))):
                    FILLER.pop(0)()

            def emit_attn(b, hp):
                smt = smt_all[b]
                smdd = smdd_all[b]

                def site_row(hp2, qi):
                    sid = hp2 * 2 + qi
                    return smt[sid // 4], 32 * (sid % 4)

                def norm_site(hp2, qi, mul_eng, mul_eng2=None, dq=None):
                    q0, qn = QW[qi]
                    sid = hp2 * 2 + qi
                    drow = 4 * (sid // 4) + (sid % 4)
                    R = rp.tile([128, 512], F32, tag="R", name=f"R{b}_{hp2}_{qi}")
                    nc.sync.dma_start(
                        out=R[0:64, :qn],
                        in_=bcast_rows(smdd[drow:drow + 1, 0:qn], 64))
                    (dq or nc.sync).dma_start(
                        out=R[64:128, :qn],
                        in_=bcast_rows(smdd[drow:drow + 1, 512:512 + qn], 64))
                    qsl_l = slice(q0, q0 + qn)
                    mul_eng.tensor_mul(xstdT[b][hp2][0:64, qsl_l],
                                       xstdT[b][hp2][0:64, qsl_l], R[0:64, :qn])
                    (mul_eng2 or mul_eng).tensor_mul(
                        xstdT[b][hp2][64:128, qsl_l],
                        xstdT[b][hp2][64:128, qsl_l], R[64:128, :qn])

                def norm_cls(hp2, mul_eng, dq=None):
                    sid = hp2 * 2
                    drow = 4 * (sid // 4) + (sid % 4)
                    Rc = rp.tile([128, 512], F32, tag="R", name=f"Rc{b}_{hp2}")
                    nc.sync.dma_start(
                        out=Rc[0:64, 0:1],
                        in_=bcast_rows(smdd[drow:drow + 1, 1024:1025], 64))
                    (dq or nc.sync).dma_start(
                        out=Rc[64:128, 0:1],
                        in_=bcast_rows(smdd[drow:drow + 1, 1025:1026], 64))
                    mul_eng.tensor_mul(xstdT[b][hp2][0:64, 1024:1025],
                                       xstdT[b][hp2][0:64, 1024:1025], Rc[0:64, 0:1])
                    mul_eng.tensor_mul(xstdT[b][hp2][64:128, 1024:1025],
                                       xstdT[b][hp2][64:128, 1024:1025], Rc[64:128, 0:1])

                qt = qk[hp]
                kt = qk[6 + hp]
                if True:
                    for q0, qn in QW:
                        psO_a = ps_o.tile([65, 512], F32, tag="psO", name=f"psOa{b}_{hp}_{q0}")
                        psO_b = ps_o.tile([65, 512], F32, tag="psO", name=f"psOb{b}_{hp}_{q0}")
                        for kc, (t0, tn) in enumerate(TCH):
                            ksl = slice(b * N + t0, b * N + t0 + tn)
                            qsl = slice(b * N + q0, b * N + q0 + qn)
                            psS = ps_s.tile([128, 1024], F32, tag="psS", name=f"psS{b}_{hp}_{q0}_{kc}")
                            # two heads row-tiled concurrently (K=64 each)
                            nc.tensor.matmul(psS[:tn, 0:qn], lhsT=kt[0:64, ksl],
                                             rhs=qt[0:64, qsl], start=True, stop=True)
                            nc.tensor.matmul(psS[:tn, 512:512 + qn], lhsT=kt[64:128, ksl],
                                             rhs=qt[64:128, qsl], start=True, stop=True)
                            pt = ptp.tile([128, 1024], BF16, tag="pt", name=f"pt{b}_{hp}_{q0}_{kc}")
                            nc.scalar.activation(pt[:tn], psS[:tn], Exp, scale=SCALE)
                            first, last = (kc == 0), (kc == 8)
                            nc.tensor.matmul(psO_a[:, :qn],
                                             lhsT=vaug[b][kc][:tn, 2 * hp * 65:2 * hp * 65 + 65],
                                             rhs=pt[:tn, 0:qn], start=first, stop=last)
                            nc.tensor.matmul(psO_b[:, :qn],
                                             lhsT=vaug[b][kc][:tn, (2 * hp + 1) * 65:(2 * hp + 1) * 65 + 65],
                                             rhs=pt[:tn, 512:512 + qn], start=first, stop=last)
                        # stash sums into the batch tile and the UNNORMALIZED
                        # O^T into xstdT (bf16); normalize in place per 2 hp.
                        st, row = site_row(hp, q0 // 512)
                        nc.vector.tensor_copy(st[row:row + 1, 0:qn], psO_a[64:65, :qn])
                        nc.vector.tensor_copy(st[row:row + 1, 512:512 + qn], psO_b[64:65, :qn])
                        qsl_l = slice(q0, q0 + qn)
                        nc.vector.tensor_copy(xstdT[b][hp][0:64, qsl_l], psO_a[0:64, :qn])
                        nc.vector.tensor_copy(xstdT[b][hp][64:128, qsl_l], psO_b[0:64, :qn])
                        # extra drains in the last head pair: the 9 proj(0)
                        # fillers must overlap attn(1,5), not pile into the tail
                        # (reserving pieces for the tail measured ~14us slower)
                        drain(4 if hp == HP - 1 else 1)

                    # ---- last query token (qtok = N-1) for this head pair ----
                    psc = ps_s.tile([128, 18], F32, tag="psS", name=f"psc{b}_{hp}")
                    nc.vector.memset(psc, 0.0)
                    # NOTE: keep hh-major. kc-major (interleaving the two
                    # row-tile bases back-to-back into the SAME single-bank
                    # psum) HANGS the device -- unlike the main S pairs,
                    # whose concurrent halves write different PSUM banks.
                    for hh in range(2):
                        hsl = slice(hh * 64, hh * 64 + 64)
                        for kc, (t0, tn) in enumerate(TCH):
                            nc.tensor.matmul(
                                psc[:tn, hh * 9 + kc: hh * 9 + kc + 1],
                                lhsT=kt[hsl, b * N + t0: b * N + t0 + tn],
                                rhs=qt[hsl, b * N + 1024: b * N + 1025],
                                start=True, stop=True,
                            )
                    ptc = ptp.tile([128, 18], BF16, tag="pt", name=f"ptc{b}_{hp}")
                    nc.scalar.activation(ptc, psc, Exp, scale=SCALE)
                    psOc_a = ps_o.tile([65, 512], F32, tag="psO", name=f"psOca{b}_{hp}")
                    psOc_b = ps_o.tile([65, 512], F32, tag="psO", name=f"psOcb{b}_{hp}")
                    for hh, psOc in ((0, psOc_a), (1, psOc_b)):
                        h = 2 * hp + hh
                        for kc, (t0, tn) in enumerate(TCH):
                            nc.tensor.matmul(
                                psOc[:, 0:1],
                                lhsT=vaug[b][kc][:tn, h * 65: h * 65 + 65],
                                rhs=ptc[:tn, hh * 9 + kc: hh * 9 + kc + 1],
                                start=(kc == 0), stop=(kc == 8),
                            )
                    st, row = site_row(hp, 0)
                    nc.vector.tensor_copy(st[row:row + 1, 1024:1025], psOc_a[64:65, 0:1])
                    nc.vector.tensor_copy(st[row:row + 1, 1025:1026], psOc_b[64:65, 0:1])
                    nc.vector.tensor_copy(xstdT[b][hp][0:64, 1024:1025], psOc_a[0:64, 0:1])
                    nc.vector.tensor_copy(xstdT[b][hp][64:128, 1024:1025], psOc_b[0:64, 0:1])
                    drain(1)

                    # ---- normalization for this smt tile (every 2nd hp) ----
                    if hp % 2 == 1:
                        t = hp // 2
                        last = hp == HP - 1
                        # 1/s = exp(-ln s) on ACT (2 x ~1.2us). The 6.7us DVE
                        # reciprocal used mid-kernel previously sat in the
                        # in-order DVE stream exactly when psO/filler psum
                        # drains queue behind it -> PE starves and the HAM
                        # drops the PE clock to 1.2GHz. The single loaded
                        # table set holds both Exp and Ln (no switch cost),
                        # and ACT has a natural dip at round boundaries.
                        def smdd_dma(c0, cn):
                            pstride = smt[t].ap[0][0]
                            nc.sync.dma_start(
                                out=smdd[4 * t:4 * t + 4, c0:c0 + cn],
                                in_=bass.AP(tensor=smt[t].tensor,
                                            offset=smt[t].offset + c0,
                                            ap=[[32 * pstride, 4], [1, cn]]),
                            )

                        def lnexp(c0, cn):
                            nc.scalar.activation(smt[t][0:97, c0:c0 + cn],
                                                 smt[t][0:97, c0:c0 + cn],
                                                 mybir.ActivationFunctionType.Ln)
                            nc.scalar.activation(smt[t][0:97, c0:c0 + cn],
                                                 smt[t][0:97, c0:c0 + cn],
                                                 Exp, scale=-1.0)

                        if last and b == 1:
                            # tail: the qw-site sums are complete ~2us before
                            # the cls sums (which trail the cls pass), and
                            # finals j=0..7 only need the qw-site norms ->
                            # recip + roundtrip + muls for cols 0:1024 start
                            # immediately; cls cols follow.
                            lnexp(0, 1024)
                            smdd_dma(0, 1024)
                        elif OPT_LNEXP:
                            lnexp(0, 1026)
                            smdd_dma(0, 1056)
                        else:
                            nc.vector.reciprocal(smt[t][0:97, :], smt[t][0:97, :])
                            smdd_dma(0, 1056)
                        if last:
                            # tail: the finals wait on ALL norm muls -> load-
                            # balance the 8 big [64,512] muls across GpSimd
                            # (~0.83us/op) and DVE (~0.4us/op): 3 on GpSimd,
                            # 5 on DVE finishes ~1us sooner than an even or
                            # per-chunk split. cls tinies ride GpSimd. For
                            # b==1 the R-broadcast dispatches (each ~0.6us of
                            # engine-queue time) ride the otherwise-idle
                            # scalar HW queue instead of serializing on sync
                            # behind the output DMAs.
                            dq = nc.scalar if b == 1 else None
                            seq = [nc.gpsimd, nc.vector, nc.vector,
                                   nc.gpsimd, nc.vector, nc.vector,
                                   nc.gpsimd, nc.vector]
                            si_ = 0
                            for hp2 in (hp - 1, hp):
                                for qi in range(2):
                                    norm_site(hp2, qi, seq[si_], seq[si_ + 1], dq=dq)
                                    si_ += 2
                            if b == 1:
                                # cls columns: recip'd after the site chain
                                # is already in flight (only final j=8, the
                                # single last-token piece, waits on these)
                                lnexp(1024, 2)
                                smdd_dma(1024, 2)
                            for hp2 in (hp - 1, hp):
                                norm_cls(hp2, nc.gpsimd, dq=dq)
                        else:
                            # mid-kernel both chunks ride GpSimd (DVE busy
                            # with drains)
                            for hp2 in (hp - 1, hp):
                                for qi in range(2):
                                    norm_site(hp2, qi, nc.gpsimd)
                                norm_cls(hp2, nc.gpsimd)

            PDONE = set()

            def emit_proj_piece(b, j):
                if (b, j) in PDONE:
                    return
                PDONE.add((b, j))
                t0, tn = TCH[j]
                if True:
                    ot = outp.tile([128, C], BF16, tag="ot", name=f"ot{b}_{j}")
                    for e0, en in [(0, 512), (512, 256)]:
                        ps = ps_lin.tile([128, 512], F32, tag="lin", name=f"psp{b}_{j}_{e0}")
                        for k in range(6):
                            nc.tensor.matmul(
                                ps[:tn, :en],
                                lhsT=xstdT[b][k][:, t0:t0 + tn],
                                rhs=pw[k][:, e0:e0 + en],
                                start=(k == 0), stop=(k == 5),
                            )
                        nc.vector.tensor_add(ot[:tn, e0:e0 + en], ps[:tn, :en], pb[:tn, e0:e0 + en])
                    nc.sync.dma_start(
                        out=out_e[b * N + t0: b * N + t0 + tn, :],
                        in_=ot[:tn, :],
                    )

            def emit_proj(b):
                for j in range(9):
                    emit_proj_piece(b, j)

            # ---- split-K proj for batch 1 (tail shortening) ----
            # proj(1,j) = sum_k xstdT[1][k].T @ pw[k]; the k=0..3 partial has
            # no dependency on the FINAL norm round (hp 4/5), so it runs as
            # round-4/5 filler, parked bf16 in dead qk slots (qk[m] for early
            # head pairs is last read in round <=3). The serial tail is then
            # only k=4..5 + a combine add. bf16 partials add ~3e-3 rel err
            # (gate is 2e-2).
            PART_SLOT = [(0, 0), (0, 1025), (1, 0), (1, 1025), (2, 0),
                         (2, 1025), (3, 0), (3, 1025), (6, 0)]

            def emit_proj1_partial(j):
                t0, tn = TCH[j]
                qi_, off = PART_SLOT[j]
                part = qk[qi_][:, off:off + C]
                for e0, en in [(0, 512), (512, 256)]:
                    ps = ps_lin.tile([128, 512], F32, tag="lin", name=f"pspa1_{j}_{e0}")
                    for k in range(4):
                        nc.tensor.matmul(
                            ps[:tn, :en],
                            lhsT=xstdT[1][k][:, t0:t0 + tn],
                            rhs=pw[k][:, e0:e0 + en],
                            start=(k == 0), stop=(k == 3),
                        )
                    # fold the bias into the partial here
                    nc.vector.tensor_add(part[:tn, e0:e0 + en], ps[:tn, :en],
                                         pb[:tn, e0:e0 + en])

            def emit_proj1_final(j):
                # attention is done by now: the ps_s pool (2x 2-bank bufs) is
                # free -- both e-chunks of a piece share ONE [128,1024] psum
                # (different banks), so there is a single combine-add per
                # piece and two pieces pipeline through the pool. Keeping
                # these off ps_lin matters: the junk screen parks a buf there.
                t0, tn = TCH[j]
                qi_, off = PART_SLOT[j]
                part = qk[qi_][:, off:off + C]
                ot = outp.tile([128, C], BF16, tag="ot", name=f"ot1_{j}")
                ps = ps_s.tile([128, 1024], F32, tag="psS", name=f"pspb1_{j}")
                for e0, en in [(0, 512), (512, 256)]:
                    for k in range(4, 6):
                        nc.tensor.matmul(
                            ps[:tn, e0:e0 + en],
                            lhsT=xstdT[1][k][:, t0:t0 + tn],
                            rhs=pw[k][:, e0:e0 + en],
                            start=(k == 4), stop=(k == 5),
                        )
                nc.vector.tensor_add(ot[:tn, 0:C], ps[:tn, 0:C], part[:tn, 0:C])
                # alternate HW queues: ACT is idle in the tail and each
                # dispatch costs ~0.6us of serial engine-queue time
                (nc.scalar if j % 2 == 0 else nc.sync).dma_start(
                    out=out_e[N + t0: N + t0 + tn, :],
                    in_=ot[:tn, :],
                )

            # ---- interleaved emission schedule ----
            # Emission order ~= static schedule priority. Attention leads;
            # LIN/proj pieces are queued as fillers drained between attention
            # stages (so they fill PE idle instead of blocking attention).
            attn_setup(0)
            attn_setup(1)
            with nc.named_scope("lin_head"):
                # window order matches DMA landing order: the first two
                # 512-windows of each head chunk need only the batch-0 xT
                # half; the rest lands while they run
                for m in (0, 6):
                    emit_linqk_piece(m, 0)
                    emit_linqk_piece(m, 512)
                for m in (0, 6):
                    for w0 in (1024, 1536, 2048):
                        emit_linqk_piece(m, w0)
                emit_linv(0)
                emit_linv(1)
            for hp in range(1, HP):
                FILLER.extend([
                    (lambda m=hp, w=w0: emit_linqk_piece(m, w))
                    for w0 in range(0, TOK, 512)
                ] + [
                    (lambda m=6 + hp, w=w0: emit_linqk_piece(m, w))
                    for w0 in range(0, TOK, 512)
                ])
            # round-boundary screens measured neutral-to-slightly-worse
            # (+1.4us): the boundary stalls are ACT-pipeline refill, not
            # idle-window throttle. Off by default.
            NJUNKR = int(os.environ.get("KJUNKR", "0"))

            def junk_screen(n, nm):
                # dependency-free wide matmuls emitted where the PE would
                # otherwise sit idle long enough for the HAM to halve the
                # clock; results are never read.
                jp = ps_lin.tile([128, 512], F32, tag="lin", name=nm)
                for i in range(n):
                    nc.tensor.matmul(jp[:, 0:512], lhsT=pw[0][:, 0:128],
                                     rhs=xT[0][:, 0:512],
                                     start=(i == 0), stop=(i == n - 1))

            emit_attn(0, 0)
            emit_attn(1, 0)
            for hp in range(1, HP):
                if NJUNKR:
                    junk_screen(NJUNKR, f"jnkr{hp}")
                while FILLER and len(FILLER) > 10 * (HP - 1 - hp):
                    FILLER.pop(0)()
                emit_attn(0, hp)
                if hp == HP - 2:
                    # proj(1) k=0..3 partials: ready (xstdT[1][0..3] was
                    # normalized after round 3) -> fill rounds 4-5
                    FILLER.extend([(lambda j=j: emit_proj1_partial(j))
                                   for j in range(9)])
                if hp == HP - 1:
                    FILLER.extend([(lambda j=j: emit_proj_piece(0, j))
                                   for j in range(9)])
                emit_attn(1, hp)
            with nc.named_scope("proj_tail"):
                while FILLER:
                    FILLER.pop(0)()
                # warm-keepers: the PE would otherwise idle ~6-8us through
                # the final norm chain (ln/exp + DRAM roundtrip + muls) and
                # the HAM would halve the clock right before the k=4..5
                # finals; a screen of dependency-free wide matmuls holds
                # activity up. Results are never read.
                NJUNK = int(os.environ.get("KJUNK", "40"))
                if NJUNK:
                    jps = ps_o.tile([65, 512], F32, tag="psO", name="jnk")
                    for i in range(NJUNK):
                        nc.tensor.matmul(
                            jps[:, 0:512],
                            lhsT=pw[0][:, 0:65],
                            rhs=xT[0][:, 0:512],
                            start=(i == 0), stop=(i == NJUNK - 1),
                        )
                for j in range(9):
                    emit_proj1_final(j)
    return nc


def _fuse_ldweights(nc):
    """Tile splits every matmul into standalone LDWEIGHTS + MATMUL; with
    this walrus build (--enable-ldw-opt=false) the pair executes serially,
    exposing ~100ns of weight-load per matmul. Re-fuse: drop the standalone
    LDW and let the matmul self-load (ldweights=True), moving any waits /
    sem updates onto the matmul (funnel pass then enforces the 1-wait cap)."""
    for f in nc.m.functions:
        for blk in f.blocks:
            insts = blk.instructions
            new = []
            pending = []  # waits/updates from deleted LDWs awaiting next MM
            changed = False
            for inst in insts:
                tn = type(inst).__name__
                if tn == "InstLdweights":
                    si = inst.sync_info
                    if si is not None and (si.on_wait or si.on_update):
                        pending.append((list(si.on_wait), list(si.on_update)))
                    changed = True
                    continue
                if tn == "InstMatmult":
                    inst.ldweights = True
                    if pending:
                        si = inst.sync_info
                        if si is None:
                            inst.sync_info = mybir.SyncInfo(on_wait=[], on_update=[])
                            si = inst.sync_info
                        w = list(si.on_wait)
                        u = list(si.on_update)
                        for pw_, pu_ in pending:
                            w.extend(pw_)
                            u.extend(pu_)
                        si.on_wait = w
                        si.on_update = u
                        pending = []
                new.append(inst)
            assert not pending, "dangling LDW sync with no following matmul"
            if changed:
                blk.instructions = new


def _funnel_pe_waits(nc):
    """Walrus allows only one sync-wait slot per engine instruction.

    Semaphores are monotonic and each engine's sequencer executes its
    stream in order, so a wait already executed by an earlier same-engine
    instruction is redundant later. Strip covered waits; if an engine
    instruction still needs >=2 waits, hoist them onto inserted
    single-wait NoOps directly before it (the sequencer executes those
    first). DMA copies / drains / event-sems use different sync hardware
    and are left untouched.
    """
    SKIP = {"InstEventSemaphore", "InstNoOp",
            "InstIncSwdgeSem", "InstTensorLoad", "InstTensorSave"}
    for f in nc.m.functions:
        for blk in f.blocks:
            insts = blk.instructions
            new = []
            seen = {e: {} for e in mybir.ALL_ENGINES}
            changed = False
            for inst in insts:
                si = getattr(inst, "sync_info", None)
                eng = inst.engine
                tn = type(inst).__name__
                if (eng in seen and tn not in SKIP
                        and si is not None and si.on_wait):
                    sn = seen[eng]
                    waits = [w for w in si.on_wait
                             if not (w.wait_mode == "sem-ge-imm"
                                     and sn.get(w.id, -1) >= w.wait_value)]
                    if tn != "InstDMACopy":
                        # DMA waits execute ring-side, not on the sequencer:
                        # they don't advance the engine's observed state
                        for w in waits:
                            if w.wait_mode == "sem-ge-imm":
                                sn[w.id] = max(sn.get(w.id, -1), w.wait_value)
                    if len(waits) > 1:
                        for wi, w in enumerate(waits):
                            noop = mybir.InstNoOp(
                                name=f"{inst.name}_wfun{wi}",
                                sync_info=mybir.SyncInfo(on_wait=[w], on_update=[]),
                                bass_nofuse=True,
                                text_hint="wait_funnel",
                            )
                            noop.engine = eng
                            new.append(noop)
                            if w.wait_mode == "sem-ge-imm":
                                sn[w.id] = max(sn.get(w.id, -1), w.wait_value)
                        waits = []
                    if len(waits) != len(si.on_wait):
                        si.on_wait = waits
                        changed = True
                new.append(inst)
            if changed or len(new) != len(insts):
                blk.instructions = new


_NC_CACHE = None


def get_nc():
    global _NC_CACHE
    if _NC_CACHE is None:
        _NC_CACHE = build_nc()
    return _NC_CACHE


def make_in_maps(x, qkv_w, proj_w, proj_b):
    bf = ml_dtypes.bfloat16
    wqkT = np.ascontiguousarray(np.asarray(qkv_w, np.float32)[:DQK].T).astype(bf)
    wvT = np.ascontiguousarray(np.asarray(qkv_w, np.float32)[DQK:].T).astype(bf)
    pwT = np.ascontiguousarray(np.asarray(proj_w, np.float32).T).astype(bf)
    pb = np.asarray(proj_b, np.float32)
    x = np.asarray(x, np.float32)
    in_maps = []
    for i in range(NCORES):
        xs = x[NB * i: NB * (i + 1)].reshape(TOK, C)
        xT = np.ascontiguousarray(xs.T).astype(bf)
        in_maps.append({"xT": xT, "wqkT": wqkT, "wvT": wvT, "pwT": pwT, "pb": pb})
    return in_maps


def _ensure_ntff_hook():
    """The agent image's antenv lacks axon_hooks; shim it so trace=True
    (profiling-only path) works instead of crashing on import."""
    import sys
    import types

    try:
        import antenv.axon_hooks  # noqa: F401
        return
    except ImportError:
        pass
    mod = types.ModuleType("antenv.axon_hooks")
    state = {"h": None}
    mod.set_axon_ntff_profile_hook = lambda h: state.__setitem__("h", h)
    mod.get_axon_ntff_profile_hook = lambda: state["h"]
    sys.modules["antenv.axon_hooks"] = mod
    import antenv

    antenv.axon_hooks = mod
    from trn_agent_boot.trn_boot import _ntff_profile_via_ctypes

    mod.set_axon_ntff_profile_hook(
        _ntff_profile_via_ctypes("/opt/axon/libaxon_pjrt.so")
    )


def kernel(x, qkv_w, proj_w, proj_b, H=None, W=None, _trace=False):
    from concourse.bass_utils import run_bass_kernel_spmd

    if _trace:
        _ensure_ntff_hook()
    nc = get_nc()
    if not getattr(nc, "_pe_waits_funneled", False):
        import os as _os
        if _os.environ.get("KFUSE_LDW", "1") == "1":
            _fuse_ldweights(nc)
        _funnel_pe_waits(nc)
        nc._pe_waits_funneled = True
    in_maps = make_in_maps(x, qkv_w, proj_w, proj_b)
    res = run_bass_kernel_spmd(nc, in_maps, core_ids=list(range(NCORES)), trace=_trace)
    out = np.concatenate(
        [r["out"].reshape(NB, N, C) for r in res.results], axis=0
    ).astype(np.float32)
    if _trace:
        kernel.last_exec_time_ns = res.exec_time_ns
        kernel.last_results = res
    return out



# revision 43
# speedup vs baseline: 1.2152x; 1.0216x over previous
"""Multi-head attention (ViT-style, N=1025 tokens incl. cls) on 8 TRN2 NeuronCores.

Reference semantics: the "separate cls-token attention" branch of the reference
is mathematically identical to row 0 of standard attention (same logits, same
softmax, same values), so the output is exactly
    out = softmax(Q K^T * hd^-0.5) V -> proj -> + bias.

Sharding: data-parallel over batch: B=16 -> 2 batches per core, weights
replicated, no collectives. ~422us HW exec on silicon (run variance
~+/-5us), rel err ~3.3e-3.

Changes vs the ~470us predecessor:
  - normalization multiplies ride the (otherwise idle) GpSimd engine
    mid-kernel, so the in-order DVE stream never blocks lin-filler psum
    drains (those stalls reset the PE DVFS p-state: a blocked PE restarts
    at 1.2GHz and needs ~3us of continuous work to regain 2.4GHz)
  - ALL normalizations compute 1/s as exp(-ln s) on ACT (2 x ~1.2us; the
    one loaded table set holds both Exp and Ln) -- the 6.7us DVE
    reciprocal sat in the in-order DVE stream exactly where psO/filler
    psum drains queue behind it, starving the PE into HAM clock drops
  - split-K proj for batch 1: the k=0..3 partial (independent of the
    final norm round) runs as round-4/5 filler, partials parked bf16 in
    dead qk slots; the serial tail is only k=4..5 + one combine-add per
    piece (phase-B psums ride the by-then-free ps_s pool: both e-chunks
    in one [128,1024] tile, one DVE add per piece)
  - input DMAs ordered by first use (the small m=0/m=6 wqk slices FIRST,
    then xT split at the batch boundary so linqk's first windows start at
    ~14us instead of ~19us; wv next, rest deferred), with dispatches
    alternating sync/scalar HW queues (each dispatch is ~0.6us of serial
    engine-queue time) and the two wqk m-slice blocks merged into one
    3D-AP DMA; lin_head emission follows DMA landing order
  - a dummy exp at t~0 pulls the ~2.7us ACT_TABLE_LOAD into the
    input-DMA window (the early-started PE otherwise waits for it at
    round 0's first exp)
  - tail R-broadcast + output-DMA dispatches alternate sync/scalar HW
    queues; the final norm's 8 big muls are load-balanced 3:5 over
    GpSimd (~0.83us/op) vs DVE (~0.4us/op)
  - a ~5us screen of dependency-free matmuls covers the tail norm chain
    so the HAM doesn't halve the PE clock right before the k=4..5 finals
  - the tail recip/roundtrip/muls are split main-cols-first / cls-cols-
    after: the qw-site sums complete ~2us before the cls sums, and only
    final j=8 (the single last-token piece) needs the cls norm
  - proj(0) fillers drain 4-per-window during the last head pair instead of
    piling into the serial tail
  - per-(b,j) proj outputs accumulate into one [128,768] tile and ship in a
    single DMA (half the output-DMA dispatch cost on the sync engine)
Failed experiments (measured SLOWER, do not retry):
  - cls-pass S matmuls kc-major (pairing the two row-tile bases into the
    same single-bank psum): DEVICE HANG -- concurrent row-tile halves
    must target different PSUM banks (main S pairs do)
  - xT DMAs split by 512-col windows: completion-sem waits coarsen to
    "#64 on this ring", first linqk waited ~25us instead of ~19
  - round-boundary junk screens (KJUNKR=3) + longer tail screen
    (KJUNK=44): +1.4us
  - fp8 DoubleRow S-matmuls: on silicon a DR matmul streams 2F moving
    elements (fold dim rides the moving stream), costing the same as the two
    bf16 row-tiled matmuls it replaces -- plus cast/fold overhead
  - balanced 9x~114 token chunks: stationary operands lose 256B alignment
    -> LDWEIGHTS penalty, ~+90us
  - software-pipelining O one chunk behind S, and JIT V-piece emission:
    both shuffle work into the DVE stream at points that block psum drains
  - reciprocal_approx_fast / gpsimd-issued DMAs: unsupported by this
    walrus/runtime (ISA-wrong-length / device hang)
  - reserving proj(0) fillers for the tail norm-chain window: +14us (x2)
  - splitting the mid-kernel DVE reciprocal into halves: +13us solo (the
    smdd DMA needs both halves, so the chain got longer, not shorter)
  - rp pool 5-6 bufs: SBUF overflow (no per-partition headroom left)
  - PE warmup junk matmuls during the input-DMA wait: neutral within noise
  - xT input DMAs on the scalar-engine queue: slightly slower

Per-core layout strategy (matmul operands bf16, f32 PSUM accumulation):
  - Host pre-transposes x / weights so contraction dims land on partitions.
  - qkT = wqkT.T @ xT      -> [1536, tok]  (Q^T,K^T: head dim on partitions)
  - V   = xT.T @ wvT       -> [tok, 768] in 65-stride head layout with a
    ones column per head (softmax sums ride the O matmul for free)
  - S^T = K_h^T.T @ Q_h^T  -> [ktok, qtok], two heads row-tiled concurrently
    (tile_position from base partitions 0/64); query windows 2x512, the last
    query column batched per head pair into a [128, 18] collector
  - P^T = exp(S^T * scale) on ScalarE, one [128, 1024] instr per k-chunk
    (ACT costs (N+352) cycles -> wide instrs; no max-subtraction needed
    since |logits| < ~4 for this distribution)
  - O^T = Vaug_h.T @ P^T   -> [65, qtok] PSUM; row 64 = softmax sums
  - unnormalized O^T is cast straight into xstdT (bf16); sums are collected
    into partition-aligned batch tiles (rows 0/32/64/96), one wide
    reciprocal per 4 sites (DVE mid-kernel, ACT exp(-ln) at the tail),
    DRAM-roundtrip partition-broadcast, then in-place multiply on GpSimd
    (no engine can broadcast across partitions; DMA can, from DRAM)
  - y = xstdT.T @ pwT + bias -> [tok, 768] -> bf16 out DMA (host casts f32)

Emission order doubles as the static-schedule priority (Tile list-scheduler):
attention leads, LIN-QK/LIN-V/proj pieces are queued as fillers drained
between attention stages so they soak up PE idle during the ACT-paced
attention pipeline.

Post-scheduling passes (this walrus allows ONE sync wait per engine
instruction): standalone LDWEIGHTS are re-fused into matmuls, then excess
waits are hoisted onto single-wait PE NoOps (semaphores are monotonic and
each sequencer executes in order, so earlier-covered waits are dropped).
"""

import os

import numpy as np
import ml_dtypes

import concourse.bass as bass
import concourse.mybir as mybir
import concourse.tile as tile

# optimization gates (bisect switches)
OPT_DMA = os.environ.get("KOPT_DMA", "1") == "1"
OPT_LNEXP = os.environ.get("KOPT_LNEXP", "1") == "1"

NCORES = 8
B, N, C = 16, 1025, 768
NB = B // NCORES          # batches per core
H = 12                    # heads
HD = C // H               # 64
HP = H // 2               # head pairs
TOK = NB * N              # tokens per core (2050)
SCALE = float(HD) ** -0.5
DQK = 2 * C               # 1536
F32 = mybir.dt.float32
BF16 = mybir.dt.bfloat16
Exp = mybir.ActivationFunctionType.Exp

# per-batch token chunks (attention / V / proj tiling): 8 x 128 + 1.
# NOTE: keep 128-chunks — a balanced 9 x ~114 split measured ~90us SLOWER
# (stationary operand offsets lose 256B alignment -> LDWEIGHTS penalty)
TCH = [(j * 128, 128) for j in range(8)] + [(1024, 1)]
# query-token windows (PSUM bank = 512 f32); last column handled in batched pass
QW = [(0, 512), (512, 512)]


def bcast_rows(ap_row, nrows):
    """AP reading one [1, n] row replicated across nrows partitions."""
    return bass.AP(
        tensor=ap_row.tensor,
        offset=ap_row.offset,
        ap=[[0, nrows]] + list(ap_row.ap[1:]),
    )


def two_blocks(ap2d, c0, w, c1):
    """3D AP over two equal-width column blocks [c0:c0+w] and [c1:c1+w] of a
    [P, ...] 2D AP -- lets one DMA cover both wqk m-slices."""
    pdim, (cstride, _) = ap2d.ap[0], ap2d.ap[1]
    return bass.AP(
        tensor=ap2d.tensor,
        offset=ap2d.offset + c0 * cstride,
        ap=[list(pdim), [(c1 - c0) * cstride, 2], [cstride, w]],
    )


def build_nc():
    nc = bass.Bass()
    xT_e = nc.declare_dram_parameter("xT", [C, TOK], BF16, isOutput=False)
    wqk_e = nc.declare_dram_parameter("wqkT", [C, DQK], BF16, isOutput=False)
    wv_e = nc.declare_dram_parameter("wvT", [C, C], BF16, isOutput=False)
    pw_e = nc.declare_dram_parameter("pwT", [C, C], BF16, isOutput=False)
    pb_e = nc.declare_dram_parameter("pb", [C], F32, isOutput=False)
    out_e = nc.declare_dram_parameter("out", [TOK, C], BF16, isOutput=True)

    with tile.TileContext(nc) as tc:
        with (
            tc.tile_pool(name="big", bufs=1) as big,
            tc.tile_pool(name="ps_lin", bufs=2, space="PSUM") as ps_lin,
            tc.tile_pool(name="ps_s", bufs=2, space="PSUM") as ps_s,
            tc.tile_pool(name="ps_o", bufs=2, space="PSUM") as ps_o,
            tc.tile_pool(name="ptp", bufs=4) as ptp,
            tc.tile_pool(name="rp", bufs=3) as rp,
            tc.tile_pool(name="smtp", bufs=6) as smtp,
            tc.tile_pool(name="dr", bufs=6, space="DRAM") as dr,
            tc.tile_pool(name="outp", bufs=3) as outp,
        ):
            # ---- persistent SBUF tensors (static: one slot per tag) ----
            def big_tile(shape, dtype, nm):
                return big.tile(shape, dtype, tag=nm, name=nm)

            xT = [big_tile([128, TOK], BF16, f"xT{k}") for k in range(6)]


            wqk = [big_tile([128, DQK], BF16, f"wqk{k}") for k in range(6)]
            wv = [big_tile([128, C], BF16, f"wv{k}") for k in range(6)]
            pw = [big_tile([128, C], BF16, f"pw{k}") for k in range(6)]
            pb = big_tile([128, C], F32, "pb")
            # Q^T|K^T chunks: m 0..5 = Q (heads 2m,2m+1), 6..11 = K
            qk = [big_tile([128, TOK], BF16, f"qk{m}") for m in range(12)]
            # V with 65-stride head layout (col 64 of each head block = ones)
            vaug = [[big_tile([128, 65 * H], BF16, f"vaug{b}_{j}")
                     for j in range(9)] for b in range(NB)]
            # attention output transposed, per c-chunk (= head pair)
            xstdT = [[big_tile([128, N], BF16, f"xstdT{b}_{k}")
                      for k in range(6)] for b in range(NB)]
            # ---- ACT warmup ----
            # walrus inserts the ~2.7us ACT_TABLE_LOAD before the FIRST
            # ACTIVATE; without this it lands on round 0's first exp, which
            # the PE (started early by the DMA ordering below) then waits
            # for. A dummy exp at t~0 pulls the load into the input-DMA
            # window.
            actw = rp.tile([128, 512], F32, tag="R", name="actwarm")
            nc.vector.memset(actw[0:1, 0:8], 1.0)
            nc.scalar.activation(actw[0:1, 0:8], actw[0:1, 0:8], Exp)

            # ---- input DMA ----
            if OPT_DMA:
                # Ordered by first use: xT split by 512-col windows (so
                # linqk(0)'s first pieces start ~7us before the last xT
                # bytes land), then just the m=0 / m=6 wqk column slices
                # (what linqk(0)/linqk(6) consume), then wv for linv,
                # deferring the remaining wqk columns (fillers, used from
                # round 1) and pw (used last).
                # NOTE: splitting xT by 512-col windows measured WORSE: the
                # DMA-completion semaphore waits coarsen to "#64 on this
                # ring" and the first linqk waited ~25us instead of ~19.
                # Each DMA dispatch costs ~0.6us on its engine queue, so the
                # ~25 input dispatches alternate between the sync and scalar
                # HW queues (ACT is idle at startup) and the two wqk m-slice
                # blocks ride one 3D-AP DMA each.
                def inq(i):
                    return nc.sync if i % 2 == 0 else nc.scalar

                # small wqk m0/m6 slices FIRST (land ~10.6us), then xT split
                # at the batch boundary: linqk(0/6)'s first two windows only
                # need the batch-0 half -> first matmul ~15us instead of
                # ~19us (the m-slices previously landed last, ~19us, because
                # they were dispatched after all 3.15MB of xT)
                for k in range(6):
                    sl = slice(k * 128, (k + 1) * 128)
                    inq(k).dma_start(out=two_blocks(wqk[k][:, :], 0, 128, 768),
                                     in_=two_blocks(wqk_e[sl, :], 0, 128, 768))
                for k in range(6):
                    sl = slice(k * 128, (k + 1) * 128)
                    inq(k).dma_start(out=xT[k][:, 0:N], in_=xT_e[sl, 0:N])
                for k in range(6):
                    sl = slice(k * 128, (k + 1) * 128)
                    inq(k).dma_start(out=xT[k][:, N:TOK], in_=xT_e[sl, N:TOK])
                for k in range(6):
                    sl = slice(k * 128, (k + 1) * 128)
                    inq(k).dma_start(out=wv[k], in_=wv_e[sl, :])
                nc.sync.dma_start(out=pb, in_=bcast_rows(pb_e[None, :], 128))
                for k in range(6):
                    sl = slice(k * 128, (k + 1) * 128)
                    inq(k).dma_start(out=two_blocks(wqk[k][:, :], 128, 640, 896),
                                     in_=two_blocks(wqk_e[sl, :], 128, 640, 896))
                for k in range(6):
                    sl = slice(k * 128, (k + 1) * 128)
                    inq(k).dma_start(out=pw[k], in_=pw_e[sl, :])
            else:
                for k in range(6):
                    sl = slice(k * 128, (k + 1) * 128)
                    nc.sync.dma_start(out=xT[k], in_=xT_e[sl, :])
                    nc.sync.dma_start(out=wqk[k], in_=wqk_e[sl, :])
                for k in range(6):
                    sl = slice(k * 128, (k + 1) * 128)
                    nc.sync.dma_start(out=wv[k], in_=wv_e[sl, :])
                nc.sync.dma_start(out=pb, in_=bcast_rows(pb_e[None, :], 128))
                for k in range(6):
                    sl = slice(k * 128, (k + 1) * 128)
                    nc.sync.dma_start(out=pw[k], in_=pw_e[sl, :])

            # ---- phase helpers (emission order = scheduling priority) ----
            def emit_linqk_piece(m, w0):
                if True:
                    wn = min(512, TOK - w0)
                    ps = ps_lin.tile([128, 512], F32, tag="lin", name=f"psqk{m}_{w0}")
                    for k in range(6):
                        nc.tensor.matmul(
                            ps[:, :wn],
                            lhsT=wqk[k][:, m * 128:(m + 1) * 128],
                            rhs=xT[k][:, w0:w0 + wn],
                            start=(k == 0), stop=(k == 5),
                        )
                    nc.vector.tensor_copy(qk[m][:, w0:w0 + wn], ps[:, :wn])

            def emit_linqk(m):
                for w0 in range(0, TOK, 512):
                    emit_linqk_piece(m, w0)

            def emit_linv_piece(b, j):
                t0, tn = TCH[j]
                if True:
                    vt = vaug[b][j]
                    for e0, en in [(0, 512), (512, 256)]:
                        ps = ps_lin.tile([128, 512], F32, tag="lin", name=f"psv{b}_{j}_{e0}")
                        for k in range(6):
                            nc.tensor.matmul(
                                ps[:tn, :en],
                                lhsT=xT[k][:, b * N + t0: b * N + t0 + tn],
                                rhs=wv[k][:, e0:e0 + en],
                                start=(k == 0), stop=(k == 5),
                            )
                        nh = en // HD
                        h0 = e0 // HD
                        dst = vt[:tn].rearrange("p (h s) -> p h s", s=65)[:, h0:h0 + nh, 0:HD]
                        src = ps[:tn, :en].rearrange("p (h s) -> p h s", s=HD)
                        nc.vector.tensor_copy(dst, src)
                    ones = vt[:tn].rearrange("p (h s) -> p h s", s=65)[:, :, HD:65]
                    nc.vector.memset(ones, 1.0)

            def emit_linv(b):
                for j in range(9):
                    emit_linv_piece(b, j)

            # ---- attention emission (per batch, per head pair) ----
            smt_all, smdd_all, site_row_all = {}, {}, {}

            def attn_setup(b):
                smt = [smtp.tile([128, 1056], F32, tag="smt", name=f"smt{b}_{t}")
                       for t in range(3)]
                for t in range(3):
                    nc.vector.memset(smt[t], 1.0)
                smt_all[b] = smt
                smdd_all[b] = dr.tile([4 * 3, 1056], F32, tag="smdd", name=f"smdd{b}")

            FILLER = []

            def drain(k):
                for _ in range(min(k, len(FILLER))):
                    FILLER.pop(0)()

            def emit_attn(b, hp):
                smt = smt_all[b]
                smdd = smdd_all[b]

                def site_row(hp2, qi):
                    sid = hp2 * 2 + qi
                    return smt[sid // 4], 32 * (sid % 4)

                def norm_site(hp2, qi, mul_eng, mul_eng2=None, dq=None):
                    q0, qn = QW[qi]
                    sid = hp2 * 2 + qi
                    drow = 4 * (sid // 4) + (sid % 4)
                    R = rp.tile([128, 512], F32, tag="R", name=f"R{b}_{hp2}_{qi}")
                    nc.sync.dma_start(
                        out=R[0:64, :qn],
                        in_=bcast_rows(smdd[drow:drow + 1, 0:qn], 64))
                    (dq or nc.sync).dma_start(
                        out=R[64:128, :qn],
                        in_=bcast_rows(smdd[drow:drow + 1, 512:512 + qn], 64))
                    qsl_l = slice(q0, q0 + qn)
                    mul_eng.tensor_mul(xstdT[b][hp2][0:64, qsl_l],
                                       xstdT[b][hp2][0:64, qsl_l], R[0:64, :qn])
                    (mul_eng2 or mul_eng).tensor_mul(
                        xstdT[b][hp2][64:128, qsl_l],
                        xstdT[b][hp2][64:128, qsl_l], R[64:128, :qn])

                def norm_cls(hp2, mul_eng, dq=None):
                    sid = hp2 * 2
                    drow = 4 * (sid // 4) + (sid % 4)
                    Rc = rp.tile([128, 512], F32, tag="R", name=f"Rc{b}_{hp2}")
                    nc.sync.dma_start(
                        out=Rc[0:64, 0:1],
                        in_=bcast_rows(smdd[drow:drow + 1, 1024:1025], 64))
                    (dq or nc.sync).dma_start(
                        out=Rc[64:128, 0:1],
                        in_=bcast_rows(smdd[drow:drow + 1, 1025:1026], 64))
                    mul_eng.tensor_mul(xstdT[b][hp2][0:64, 1024:1025],
                                       xstdT[b][hp2][0:64, 1024:1025], Rc[0:64, 0:1])
                    mul_eng.tensor_mul(xstdT[b][hp2][64:128, 1024:1025],
                                       xstdT[b][hp2][64:128, 1024:1025], Rc[64:128, 0:1])

                qt = qk[hp]
                kt = qk[6 + hp]
                if True:
                    for q0, qn in QW:
                        psO_a = ps_o.tile([65, 512], F32, tag="psO", name=f"psOa{b}_{hp}_{q0}")
                        psO_b = ps_o.tile([65, 512], F32, tag="psO", name=f"psOb{b}_{hp}_{q0}")
                        for kc, (t0, tn) in enumerate(TCH):
                            ksl = slice(b * N + t0, b * N + t0 + tn)
                            qsl = slice(b * N + q0, b * N + q0 + qn)
                            psS = ps_s.tile([128, 1024], F32, tag="psS", name=f"psS{b}_{hp}_{q0}_{kc}")
                            # two heads row-tiled concurrently (K=64 each)
                            nc.tensor.matmul(psS[:tn, 0:qn], lhsT=kt[0:64, ksl],
                                             rhs=qt[0:64, qsl], start=True, stop=True)
                            nc.tensor.matmul(psS[:tn, 512:512 + qn], lhsT=kt[64:128, ksl],
                                             rhs=qt[64:128, qsl], start=True, stop=True)
                            pt = ptp.tile([128, 1024], BF16, tag="pt", name=f"pt{b}_{hp}_{q0}_{kc}")
                            nc.scalar.activation(pt[:tn], psS[:tn], Exp, scale=SCALE)
                            first, last = (kc == 0), (kc == 8)
                            nc.tensor.matmul(psO_a[:, :qn],
                                             lhsT=vaug[b][kc][:tn, 2 * hp * 65:2 * hp * 65 + 65],
                                             rhs=pt[:tn, 0:qn], start=first, stop=last)
                            nc.tensor.matmul(psO_b[:, :qn],
                                             lhsT=vaug[b][kc][:tn, (2 * hp + 1) * 65:(2 * hp + 1) * 65 + 65],
                                             rhs=pt[:tn, 512:512 + qn], start=first, stop=last)
                        # stash sums into the batch tile and the UNNORMALIZED
                        # O^T into xstdT (bf16); normalize in place per 2 hp.
                        st, row = site_row(hp, q0 // 512)
                        nc.vector.tensor_copy(st[row:row + 1, 0:qn], psO_a[64:65, :qn])
                        nc.vector.tensor_copy(st[row:row + 1, 512:512 + qn], psO_b[64:65, :qn])
                        qsl_l = slice(q0, q0 + qn)
                        nc.vector.tensor_copy(xstdT[b][hp][0:64, qsl_l], psO_a[0:64, :qn])
                        nc.vector.tensor_copy(xstdT[b][hp][64:128, qsl_l], psO_b[0:64, :qn])
                        # extra drains in the last head pair: the 9 proj(0)
                        # fillers must overlap attn(1,5), not pile into the tail
                        # (reserving pieces for the tail measured ~14us slower)
                        drain(4 if hp == HP - 1 else 1)

                    # ---- last query token (qtok = N-1) for this head pair ----
                    psc = ps_s.tile([128, 18], F32, tag="psS", name=f"psc{b}_{hp}")
                    nc.vector.memset(psc, 0.0)
                    # NOTE: keep hh-major. kc-major (interleaving the two
                    # row-tile bases back-to-back into the SAME single-bank
                    # psum) HANGS the device -- unlike the main S pairs,
                    # whose concurrent halves write different PSUM banks.
                    for hh in range(2):
                        hsl = slice(hh * 64, hh * 64 + 64)
                        for kc, (t0, tn) in enumerate(TCH):
                            nc.tensor.matmul(
                                psc[:tn, hh * 9 + kc: hh * 9 + kc + 1],
                                lhsT=kt[hsl, b * N + t0: b * N + t0 + tn],
                                rhs=qt[hsl, b * N + 1024: b * N + 1025],
                                start=True, stop=True,
                            )
                    ptc = ptp.tile([128, 18], BF16, tag="pt", name=f"ptc{b}_{hp}")
                    nc.scalar.activation(ptc, psc, Exp, scale=SCALE)
                    psOc_a = ps_o.tile([65, 512], F32, tag="psO", name=f"psOca{b}_{hp}")
                    psOc_b = ps_o.tile([65, 512], F32, tag="psO", name=f"psOcb{b}_{hp}")
                    for hh, psOc in ((0, psOc_a), (1, psOc_b)):
                        h = 2 * hp + hh
                        for kc, (t0, tn) in enumerate(TCH):
                            nc.tensor.matmul(
                                psOc[:, 0:1],
                                lhsT=vaug[b][kc][:tn, h * 65: h * 65 + 65],
                                rhs=ptc[:tn, hh * 9 + kc: hh * 9 + kc + 1],
                                start=(kc == 0), stop=(kc == 8),
                            )
                    st, row = site_row(hp, 0)
                    nc.vector.tensor_copy(st[row:row + 1, 1024:1025], psOc_a[64:65, 0:1])
                    nc.vector.tensor_copy(st[row:row + 1, 1025:1026], psOc_b[64:65, 0:1])
                    nc.vector.tensor_copy(xstdT[b][hp][0:64, 1024:1025], psOc_a[0:64, 0:1])
                    nc.vector.tensor_copy(xstdT[b][hp][64:128, 1024:1025], psOc_b[0:64, 0:1])
                    drain(1)

                    # ---- normalization for this smt tile (every 2nd hp) ----
                    if hp % 2 == 1:
                        t = hp // 2
                        last = hp == HP - 1
                        # 1/s = exp(-ln s) on ACT (2 x ~1.2us). The 6.7us DVE
                        # reciprocal used mid-kernel previously sat in the
                        # in-order DVE stream exactly when psO/filler psum
                        # drains queue behind it -> PE starves and the HAM
                        # drops the PE clock to 1.2GHz. The single loaded
                        # table set holds both Exp and Ln (no switch cost),
                        # and ACT has a natural dip at round boundaries.
                        def smdd_dma(c0, cn):
                            pstride = smt[t].ap[0][0]
                            nc.sync.dma_start(
                                out=smdd[4 * t:4 * t + 4, c0:c0 + cn],
                                in_=bass.AP(tensor=smt[t].tensor,
                                            offset=smt[t].offset + c0,
                                            ap=[[32 * pstride, 4], [1, cn]]),
                            )

                        def lnexp(c0, cn):
                            nc.scalar.activation(smt[t][0:97, c0:c0 + cn],
                                                 smt[t][0:97, c0:c0 + cn],
                                                 mybir.ActivationFunctionType.Ln)
                            nc.scalar.activation(smt[t][0:97, c0:c0 + cn],
                                                 smt[t][0:97, c0:c0 + cn],
                                                 Exp, scale=-1.0)

                        if last and b == 1:
                            # tail: the qw-site sums are complete ~2us before
                            # the cls sums (which trail the cls pass), and
                            # finals j=0..7 only need the qw-site norms ->
                            # recip + roundtrip + muls for cols 0:1024 start
                            # immediately; cls cols follow.
                            lnexp(0, 1024)
                            smdd_dma(0, 1024)
                        elif OPT_LNEXP:
                            lnexp(0, 1026)
                            smdd_dma(0, 1056)
                        else:
                            nc.vector.reciprocal(smt[t][0:97, :], smt[t][0:97, :])
                            smdd_dma(0, 1056)
                        if last:
                            # tail: the finals wait on ALL norm muls -> load-
                            # balance the 8 big [64,512] muls across GpSimd
                            # (~0.83us/op) and DVE (~0.4us/op): 3 on GpSimd,
                            # 5 on DVE finishes ~1us sooner than an even or
                            # per-chunk split. cls tinies ride GpSimd. For
                            # b==1 the R-broadcast dispatches (each ~0.6us of
                            # engine-queue time) ride the otherwise-idle
                            # scalar HW queue instead of serializing on sync
                            # behind the output DMAs.
                            dq = nc.scalar if b == 1 else None
                            seq = [nc.gpsimd, nc.vector, nc.vector,
                                   nc.gpsimd, nc.vector, nc.vector,
                                   nc.gpsimd, nc.vector]
                            si_ = 0
                            for hp2 in (hp - 1, hp):
                                for qi in range(2):
                                    norm_site(hp2, qi, seq[si_], seq[si_ + 1], dq=dq)
                                    si_ += 2
                            if b == 1:
                                # cls columns: recip'd after the site chain
                                # is already in flight (only final j=8, the
                                # single last-token piece, waits on these)
                                lnexp(1024, 2)
                                smdd_dma(1024, 2)
                            for hp2 in (hp - 1, hp):
                                norm_cls(hp2, nc.gpsimd, dq=dq)
                        else:
                            # mid-kernel both chunks ride GpSimd (DVE busy
                            # with drains)
                            for hp2 in (hp - 1, hp):
                                for qi in range(2):
                                    norm_site(hp2, qi, nc.gpsimd)
                                norm_cls(hp2, nc.gpsimd)

            PDONE = set()

            def emit_proj_piece(b, j):
                if (b, j) in PDONE:
                    return
                PDONE.add((b, j))
                t0, tn = TCH[j]
                if True:
                    ot = outp.tile([128, C], BF16, tag="ot", name=f"ot{b}_{j}")
                    for e0, en in [(0, 512), (512, 256)]:
                        ps = ps_lin.tile([128, 512], F32, tag="lin", name=f"psp{b}_{j}_{e0}")
                        for k in range(6):
                            nc.tensor.matmul(
                                ps[:tn, :en],
                                lhsT=xstdT[b][k][:, t0:t0 + tn],
                                rhs=pw[k][:, e0:e0 + en],
                                start=(k == 0), stop=(k == 5),
                            )
                        nc.vector.tensor_add(ot[:tn, e0:e0 + en], ps[:tn, :en], pb[:tn, e0:e0 + en])
                    nc.sync.dma_start(
                        out=out_e[b * N + t0: b * N + t0 + tn, :],
                        in_=ot[:tn, :],
                    )

            def emit_proj(b):
                for j in range(9):
                    emit_proj_piece(b, j)

            # ---- split-K proj for batch 1 (tail shortening) ----
            # proj(1,j) = sum_k xstdT[1][k].T @ pw[k]; the k=0..3 partial has
            # no dependency on the FINAL norm round (hp 4/5), so it runs as
            # round-4/5 filler, parked bf16 in dead qk slots (qk[m] for early
            # head pairs is last read in round <=3). The serial tail is then
            # only k=4..5 + a combine add. bf16 partials add ~3e-3 rel err
            # (gate is 2e-2).
            PART_SLOT = [(0, 0), (0, 1025), (1, 0), (1, 1025), (2, 0),
                         (2, 1025), (3, 0), (3, 1025), (6, 0)]

            def emit_proj1_partial(j):
                t0, tn = TCH[j]
                qi_, off = PART_SLOT[j]
                part = qk[qi_][:, off:off + C]
                for e0, en in [(0, 512), (512, 256)]:
                    ps = ps_lin.tile([128, 512], F32, tag="lin", name=f"pspa1_{j}_{e0}")
                    for k in range(4):
                        nc.tensor.matmul(
                            ps[:tn, :en],
                            lhsT=xstdT[1][k][:, t0:t0 + tn],
                            rhs=pw[k][:, e0:e0 + en],
                            start=(k == 0), stop=(k == 3),
                        )
                    # fold the bias into the partial here
                    nc.vector.tensor_add(part[:tn, e0:e0 + en], ps[:tn, :en],
                                         pb[:tn, e0:e0 + en])

            def emit_proj1_final(j):
                # attention is done by now: the ps_s pool (2x 2-bank bufs) is
                # free -- both e-chunks of a piece share ONE [128,1024] psum
                # (different banks), so there is a single combine-add per
                # piece and two pieces pipeline through the pool. Keeping
                # these off ps_lin matters: the junk screen parks a buf there.
                t0, tn = TCH[j]
                qi_, off = PART_SLOT[j]
                part = qk[qi_][:, off:off + C]
                ot = outp.tile([128, C], BF16, tag="ot", name=f"ot1_{j}")
                ps = ps_s.tile([128, 1024], F32, tag="psS", name=f"pspb1_{j}")
                for e0, en in [(0, 512), (512, 256)]:
                    for k in range(4, 6):
                        nc.tensor.matmul(
                            ps[:tn, e0:e0 + en],
                            lhsT=xstdT[1][k][:, t0:t0 + tn],
                            rhs=pw[k][:, e0:e0 + en],
                            start=(k == 4), stop=(k == 5),
                        )
                nc.vector.tensor_add(ot[:tn, 0:C], ps[:tn, 0:C], part[:tn, 0:C])
                # alternate HW queues: ACT is idle in the tail and each
                # dispatch costs ~0.6us of serial engine-queue time
                (nc.scalar if j % 2 == 0 else nc.sync).dma_start(
                    out=out_e[N + t0: N + t0 + tn, :],
                    in_=ot[:tn, :],
                )

            # ---- interleaved emission schedule ----
            # Emission order ~= static schedule priority. Attention leads;
            # LIN/proj pieces are queued as fillers drained between attention
            # stages (so they fill PE idle instead of blocking attention).
            attn_setup(0)
            attn_setup(1)
            with nc.named_scope("lin_head"):
                # window order matches DMA landing order: the first two
                # 512-windows of each head chunk need only the batch-0 xT
                # half; the rest lands while they run
                for m in (0, 6):
                    emit_linqk_piece(m, 0)
                    emit_linqk_piece(m, 512)
                for m in (0, 6):
                    for w0 in (1024, 1536, 2048):
                        emit_linqk_piece(m, w0)
                emit_linv(0)
                emit_linv(1)
            for hp in range(1, HP):
                FILLER.extend([
                    (lambda m=hp, w=w0: emit_linqk_piece(m, w))
                    for w0 in range(0, TOK, 512)
                ] + [
                    (lambda m=6 + hp, w=w0: emit_linqk_piece(m, w))
                    for w0 in range(0, TOK, 512)
                ])
            # round-boundary screens measured neutral-to-slightly-worse
            # (+1.4us): the boundary stalls are ACT-pipeline refill, not
            # idle-window throttle. Off by default.
            NJUNKR = int(os.environ.get("KJUNKR", "0"))

            def junk_screen(n, nm):
                # dependency-free wide matmuls emitted where the PE would
                # otherwise sit idle long enough for the HAM to halve the
                # clock; results are never read.
                jp = ps_lin.tile([128, 512], F32, tag="lin", name=nm)
                for i in range(n):
                    nc.tensor.matmul(jp[:, 0:512], lhsT=pw[0][:, 0:128],
                                     rhs=xT[0][:, 0:512],
                                     start=(i == 0), stop=(i == n - 1))

            emit_attn(0, 0)
            emit_attn(1, 0)
            for hp in range(1, HP):
                if NJUNKR:
                    junk_screen(NJUNKR, f"jnkr{hp}")
                while FILLER and len(FILLER) > 10 * (HP - 1 - hp):
                    FILLER.pop(0)()
                emit_attn(0, hp)
                if hp == HP - 2:
                    # proj(1) k=0..3 partials: ready (xstdT[1][0..3] was
                    # normalized after round 3) -> fill rounds 4-5
                    FILLER.extend([(lambda j=j: emit_proj1_partial(j))
                                   for j in range(9)])
                if hp == HP - 1:
                    FILLER.extend([(lambda j=j: emit_proj_piece(0, j))
                                   for j in range(9)])
                emit_attn(1, hp)
            with nc.named_scope("proj_tail"):
                while FILLER:
                    FILLER.pop(0)()
                # warm-keepers: the PE would otherwise idle ~6-8us through
                # the final norm chain (ln/exp + DRAM roundtrip + muls) and
                # the HAM would halve the clock right before the k=4..5
                # finals; a screen of dependency-free wide matmuls holds
                # activity up. Results are never read.
                NJUNK = int(os.environ.get("KJUNK", "40"))
                if NJUNK:
                    jps = ps_o.tile([65, 512], F32, tag="psO", name="jnk")
                    for i in range(NJUNK):
                        nc.tensor.matmul(
                            jps[:, 0:512],
                            lhsT=pw[0][:, 0:65],
                            rhs=xT[0][:, 0:512],
                            start=(i == 0), stop=(i == NJUNK - 1),
                        )
                for j in range(9):
                    emit_proj1_final(j)
    return nc


def _fuse_ldweights(nc):
    """Tile splits every matmul into standalone LDWEIGHTS + MATMUL; with
    this walrus build (--enable-ldw-opt=false) the pair executes serially,
    exposing ~100ns of weight-load per matmul. Re-fuse: drop the standalone
    LDW and let the matmul self-load (ldweights=True), moving any waits /
    sem updates onto the matmul (funnel pass then enforces the 1-wait cap)."""
    for f in nc.m.functions:
        for blk in f.blocks:
            insts = blk.instructions
            new = []
            pending = []  # waits/updates from deleted LDWs awaiting next MM
            changed = False
            for inst in insts:
                tn = type(inst).__name__
                if tn == "InstLdweights":
                    si = inst.sync_info
                    if si is not None and (si.on_wait or si.on_update):
                        pending.append((list(si.on_wait), list(si.on_update)))
                    changed = True
                    continue
                if tn == "InstMatmult":
                    inst.ldweights = True
                    if pending:
                        si = inst.sync_info
                        if si is None:
                            inst.sync_info = mybir.SyncInfo(on_wait=[], on_update=[])
                            si = inst.sync_info
                        w = list(si.on_wait)
                        u = list(si.on_update)
                        for pw_, pu_ in pending:
                            w.extend(pw_)
                            u.extend(pu_)
                        si.on_wait = w
                        si.on_update = u
                        pending = []
                new.append(inst)
            assert not pending, "dangling LDW sync with no following matmul"
            if changed:
                blk.instructions = new


def _funnel_pe_waits(nc):
    """Walrus allows only one sync-wait slot per engine instruction.

    Semaphores are monotonic and each engine's sequencer executes its
    stream in order, so a wait already executed by an earlier same-engine
    instruction is redundant later. Strip covered waits; if an engine
    instruction still needs >=2 waits, hoist them onto inserted
    single-wait NoOps directly before it (the sequencer executes those
    first). DMA copies / drains / event-sems use different sync hardware
    and are left untouched.
    """
    SKIP = {"InstEventSemaphore", "InstNoOp",
            "InstIncSwdgeSem", "InstTensorLoad", "InstTensorSave"}
    for f in nc.m.functions:
        for blk in f.blocks:
            insts = blk.instructions
            new = []
            seen = {e: {} for e in mybir.ALL_ENGINES}
            changed = False
            for inst in insts:
                si = getattr(inst, "sync_info", None)
                eng = inst.engine
                tn = type(inst).__name__
                if (eng in seen and tn not in SKIP
                        and si is not None and si.on_wait):
                    sn = seen[eng]
                    waits = [w for w in si.on_wait
                             if not (w.wait_mode == "sem-ge-imm"
                                     and sn.get(w.id, -1) >= w.wait_value)]
                    if tn != "InstDMACopy":
                        # DMA waits execute ring-side, not on the sequencer:
                        # they don't advance the engine's observed state
                        for w in waits:
                            if w.wait_mode == "sem-ge-imm":
                                sn[w.id] = max(sn.get(w.id, -1), w.wait_value)
                    if len(waits) > 1:
                        for wi, w in enumerate(waits):
                            noop = mybir.InstNoOp(
                                name=f"{inst.name}_wfun{wi}",
                                sync_info=mybir.SyncInfo(on_wait=[w], on_update=[]),
                                bass_nofuse=True,
                                text_hint="wait_funnel",
                            )
                            noop.engine = eng
                            new.append(noop)
                            if w.wait_mode == "sem-ge-imm":
                                sn[w.id] = max(sn.get(w.id, -1), w.wait_value)
                        waits = []
                    if len(waits) != len(si.on_wait):
                        si.on_wait = waits
                        changed = True
                new.append(inst)
            if changed or len(new) != len(insts):
                blk.instructions = new


_NC_CACHE = None


def get_nc():
    global _NC_CACHE
    if _NC_CACHE is None:
        _NC_CACHE = build_nc()
    return _NC_CACHE


def make_in_maps(x, qkv_w, proj_w, proj_b):
    bf = ml_dtypes.bfloat16
    wqkT = np.ascontiguousarray(np.asarray(qkv_w, np.float32)[:DQK].T).astype(bf)
    wvT = np.ascontiguousarray(np.asarray(qkv_w, np.float32)[DQK:].T).astype(bf)
    pwT = np.ascontiguousarray(np.asarray(proj_w, np.float32).T).astype(bf)
    pb = np.asarray(proj_b, np.float32)
    x = np.asarray(x, np.float32)
    in_maps = []
    for i in range(NCORES):
        xs = x[NB * i: NB * (i + 1)].reshape(TOK, C)
        xT = np.ascontiguousarray(xs.T).astype(bf)
        in_maps.append({"xT": xT, "wqkT": wqkT, "wvT": wvT, "pwT": pwT, "pb": pb})
    return in_maps


def _ensure_ntff_hook():
    """The agent image's antenv lacks axon_hooks; shim it so trace=True
    (profiling-only path) works instead of crashing on import."""
    import sys
    import types

    try:
        import antenv.axon_hooks  # noqa: F401
        return
    except ImportError:
        pass
    mod = types.ModuleType("antenv.axon_hooks")
    state = {"h": None}
    mod.set_axon_ntff_profile_hook = lambda h: state.__setitem__("h", h)
    mod.get_axon_ntff_profile_hook = lambda: state["h"]
    sys.modules["antenv.axon_hooks"] = mod
    import antenv

    antenv.axon_hooks = mod
    from trn_agent_boot.trn_boot import _ntff_profile_via_ctypes

    mod.set_axon_ntff_profile_hook(
        _ntff_profile_via_ctypes("/opt/axon/libaxon_pjrt.so")
    )


def kernel(x, qkv_w, proj_w, proj_b, H=None, W=None, _trace=False):
    from concourse.bass_utils import run_bass_kernel_spmd

    if _trace:
        _ensure_ntff_hook()
    nc = get_nc()
    if not getattr(nc, "_pe_waits_funneled", False):
        import os as _os
        if _os.environ.get("KFUSE_LDW", "1") == "1":
            _fuse_ldweights(nc)
        _funnel_pe_waits(nc)
        nc._pe_waits_funneled = True
    in_maps = make_in_maps(x, qkv_w, proj_w, proj_b)
    res = run_bass_kernel_spmd(nc, in_maps, core_ids=list(range(NCORES)), trace=_trace)
    out = np.concatenate(
        [r["out"].reshape(NB, N, C) for r in res.results], axis=0
    ).astype(np.float32)
    if _trace:
        kernel.last_exec_time_ns = res.exec_time_ns
        kernel.last_results = res
    return out

